# revision 1
# baseline (speedup 1.0000x reference)
"""Trainium2 Bass kernel for nn_CRM_14886356648008 (LIIF-style SR module).

Sharding: 8 cores = 4 images x 2 output-row halves. Each core gets the full
input image plus precomputed static tables and computes out[b, :, h*128:(h+1)*128, :].

Per-core algorithm (per 2-output-row chunk of 512 queries):
  1. First (regular-grid) bicubic sample as two static banded matmuls.
  2. 7-layer MLP stack (pred / routing / offset heads) as PE matmuls.
  3. Offset bicubic sample: per-oy banded matrix built by evaluating the
     piecewise cubic kernel K(|w - ix2|) densely on [128 w, 512] via a
     branchless clamp form; vertical direction via 8 compact candidate
     weights (sentinel 1e4 row index => weight exactly 0 for OOB taps),
     broadcast with a tiled-identity matmul, multiplied and group-summed
     back to [32 c, 256 ox] with static selector matmuls.
"""
import os
import numpy as np

import concourse.bass as bass
import concourse.tile as tile
from concourse import mybir
from concourse.bass_utils import run_bass_kernel_spmd

F32 = mybir.dt.float32
F32R = mybir.dt.float32r
AF = mybir.ActivationFunctionType
ALU = mybir.AluOpType

A = -0.75
B, C, H, W = 4, 32, 128, 128
SCALE = 2
HO, WO = H * SCALE, W * SCALE
J = 76        # x_loc free rows per channel
SHIFT = 7     # x_loc[j] = image row j + 64*h - SHIFT
N_CORES = 8
OYS = 128     # output rows per core
N_GROUPS = int(os.environ.get("KGROUPS", "8"))   # 16 oys per group
USE_F32R = os.environ.get("KF32R", "1") == "1"

# ----------------------------------------------------------------------------
# host-side reference math (for the static rel-coord tables)
# ----------------------------------------------------------------------------

def _cubic_weights(t):
    x = t + 1.0
    w0 = ((A * x - 5.0 * A) * x + 8.0 * A) * x - 4.0 * A
    w1 = ((A + 2.0) * t - (A + 3.0)) * t * t + 1.0
    s = 1.0 - t
    w2 = ((A + 2.0) * s - (A + 3.0)) * s * s + 1.0
    w3 = 1.0 - w0 - w1 - w2
    return np.stack([w0, w1, w2, w3], axis=-1)


def _grid_sample_bicubic_np(feat, gx, gy):
    Bn, Cn, Hn, Wn = feat.shape
    ix = ((gx + 1.0) * Wn - 1.0) * 0.5
    iy = ((gy + 1.0) * Hn - 1.0) * 0.5
    ix0 = np.floor(ix)
    iy0 = np.floor(iy)
    wx = _cubic_weights(ix - ix0)
    wy = _cubic_weights(iy - iy0)
    ix0 = ix0.astype(np.int32)
    iy0 = iy0.astype(np.int32)
    ff = feat.reshape(Bn, Cn, Hn * Wn)
    out = np.zeros((Bn, Cn, gx.shape[1]), feat.dtype)
    for i in range(4):
        yi = iy0 - 1 + i
        yok = (yi >= 0) & (yi < Hn)
        yc = np.clip(yi, 0, Hn - 1)
        for jj in range(4):
            xj = ix0 - 1 + jj
            ok = yok & (xj >= 0) & (xj < Wn)
            xc = np.clip(xj, 0, Wn - 1)
            v = np.take_along_axis(ff, (yc * Wn + xc)[:, None, :], axis=2)
            w = wy[..., i] * wx[..., jj] * ok
            out = out + v * w[:, None, :].astype(feat.dtype)
    return out


def _make_coord(Hn, Wn):
    y = -1.0 + (2.0 * np.arange(Hn, dtype=np.float32) + 1.0) / Hn
    x = -1.0 + (2.0 * np.arange(Wn, dtype=np.float32) + 1.0) / Wn
    yy, xx = np.meshgrid(y, x, indexing="ij")
    return np.stack([yy, xx], axis=-1).astype(np.float32)


def _rel_table():
    """rel features [Q, 4] = [rel_coord_y, rel_coord_x, rel_cell_y, rel_cell_x]."""
    Q = HO * WO
    coord = _make_coord(HO, WO).reshape(Q, 2)
    cell = np.ones((Q, 2), np.float32)
    cell[0] *= 2.0 / HO
    cell[1] *= 2.0 / WO
    cc = np.clip(coord, -1.0 + 1e-6, 1.0 - 1e-6)
    gy, gx = cc[None, :, 0], cc[None, :, 1]
    fc = np.broadcast_to(_make_coord(H, W).transpose(2, 0, 1)[None], (1, 2, H, W))
    q_coord = _grid_sample_bicubic_np(fc, gx, gy).transpose(0, 2, 1)[0]
    hw = np.array([H, W], np.float32)
    rel_coord = (coord - q_coord) * hw
    rel_cell = cell * hw
    return np.concatenate([rel_coord, rel_cell], axis=-1).astype(np.float32)


def _ky_l(oy_l):
    return int(np.floor(oy_l / 2.0 - 0.25))


def _s_prog(oy_l):
    return _ky_l(oy_l) + 4


# ----------------------------------------------------------------------------
# per-core input tables
# ----------------------------------------------------------------------------

def _build_core_inputs(inputs, b, h, shared):
    x = np.asarray(inputs["x"])[b]
    # x_loc[w, j*C + c] = x[c, j + 64h - SHIFT, w]  (j-major so any 8-row
    # band slice is a contiguous 2D stationary AP)
    x_loc = np.zeros((W, J, C), np.float32)
    rows = np.arange(J) + 64 * h - SHIFT
    valid = (rows >= 0) & (rows < H)
    x_loc[:, valid, :] = x[:, rows[valid], :].transpose(2, 1, 0)

    # first-sample vertical lhsT per oy: M-partition k = r*32 + c
    v0all = np.zeros((128, OYS * 32), np.float32)
    for oy_l in range(OYS):
        oy_g = h * 128 + oy_l
        iy = oy_g / 2.0 - 0.25
        ky = int(np.floor(iy))
        wts = _cubic_weights(np.float32(iy - ky))
        for r in range(4):
            row = ky - 1 + r
            if 0 <= row < H:
                for c in range(C):
                    v0all[r * 32 + c, oy_l * 32 + c] = wts[r]

    b_off = np.asarray(inputs["off_b1"], np.float32)
    # candc2[(o,r), g] = candpos - iy(oy) - 64*b_off_y  (sentinel 1e4 for OOB
    # rows -> |d| >= 2 -> weight exactly 0)
    candc = np.zeros((128, 8), np.float32)
    for g in range(8):
        for o in range(16):
            oy_g = h * 128 + 16 * g + o
            iy = oy_g / 2.0 - 0.25
            ky = int(np.floor(iy))
            for r in range(8):
                row = ky - 3 + r
                candc[o * 8 + r, g] = (
                    row - iy - 64.0 * b_off[1] if 0 <= row < H else 1e4)

    q0 = h * 128 * 256
    relrows = np.ascontiguousarray(shared["rel"][q0:q0 + OYS * 256].T)

    # dstat[w, t*256+ox] = w - ix(ox) - 64*b_off_x
    ox = np.arange(256, dtype=np.float32)
    ixb = ox / 2.0 - 0.25 + 64.0 * b_off[0]
    dcol = np.arange(W, dtype=np.float32)[:, None] - ixb[None, :]
    dstat = np.tile(dcol, (1, 2)).astype(np.float32)

    d = {
        "x_loc": np.ascontiguousarray(x_loc.reshape(W, C * J)),
        "v0all": v0all,
        "candc": candc,
        "relrows": relrows,
        "dstat": np.ascontiguousarray(dstat),
    }
    d.update(shared["static"])
    return d


def _pack_off_w1(ow):  # [256, 2] -> [128, 128]: cols kh*64+{0,32}
    ow = np.asarray(ow, np.float32)
    out = np.zeros((128, 128), np.float32)
    for kh in range(2):
        out[:, kh * 64 + 0] = ow[kh * 128:(kh + 1) * 128, 0]
        out[:, kh * 64 + 32] = ow[kh * 128:(kh + 1) * 128, 1]
    return out


def _build_shared(inputs):
    rel = _rel_table()

    bandx0 = np.zeros((W, WO), np.float32)
    for ox in range(WO):
        ix = ox / 2.0 - 0.25
        kx = int(np.floor(ix))
        wts = _cubic_weights(np.float32(ix - kx))
        for tap in range(4):
            wc = kx - 1 + tap
            if 0 <= wc < W:
                bandx0[wc, ox] = wts[tap]

    def pack_k(wm):  # [256, M] -> [128, 2*M]
        wm = np.asarray(wm, np.float32)
        M = wm.shape[1]
        out = np.zeros((128, 2 * M), np.float32)
        out[:, :M] = wm[:128]
        out[:, M:] = wm[128:]
        return out

    def pack_b(bv):  # [256] -> [128, 2]
        bv = np.asarray(bv, np.float32)
        return np.ascontiguousarray(np.stack([bv[:128], bv[128:]], axis=1))

    # gather M-partition ordering is (r, c): m = r*32 + c, split into halves
    # r 0-3 (id8a) and r 4-7 (id8b); channel group-sum is shared (sumc).
    id8a = np.zeros((8, 128), np.float32)
    id8a[np.arange(128) // 32, np.arange(128)] = 1.0
    id8b = np.zeros((8, 128), np.float32)
    id8b[4 + np.arange(128) // 32, np.arange(128)] = 1.0
    oybcast = np.zeros((16, 128), np.float32)
    oybcast[np.arange(128) // 8, np.arange(128)] = 1.0
    sumc = np.zeros((128, 32), np.float32)
    sumc[np.arange(128), np.arange(128) % 32] = 1.0

    static = {
        "bandx0": bandx0,
        "w0": np.asarray(inputs["mlp_w0"], np.float32),
        "b0": pack_b(inputs["mlp_b0"]),
        "w1p": pack_k(inputs["mlp_w1"]),
        "b1": pack_b(inputs["mlp_b1"]),
        "w2p": pack_k(inputs["mlp_w2"]),
        "b2": np.asarray(inputs["mlp_b2"], np.float32).reshape(32, 1),
        "rw0": np.asarray(inputs["rout_w0"], np.float32),
        "rb0": pack_b(inputs["rout_b0"]),
        "rw1p": pack_k(inputs["rout_w1"]),
        "rb1p1": (np.asarray(inputs["rout_b1"], np.float32) + 1.0).reshape(32, 1),
        "ow0": np.asarray(inputs["off_w0"], np.float32),
        "ob0": pack_b(inputs["off_b0"]),
        # off head: off_x -> out partition 0, off_y -> out partition 32
        # (DVE operand partition bases must be 32-aligned)
        "ow1p": _pack_off_w1(inputs["off_w1"]),
        "id8a": id8a,
        "id8b": id8b,
        "oybcast": oybcast,
        "ones1": np.ones((1, 128), np.float32),
        "neg_iota": np.ascontiguousarray(-np.arange(128, dtype=np.float32).reshape(128, 1)),
        "sumc": sumc,
    }
    return {"rel": rel, "static": static}


# ----------------------------------------------------------------------------
# the Bass program (shared SPMD; per-core differences flow via DRAM data)
# ----------------------------------------------------------------------------

_SHAPES = {
    "x_loc": (W, C * J), "bandx0": (W, WO), "v0all": (128, OYS * 32),
    "candc": (128, 8), "relrows": (4, OYS * 256), "dstat": (128, 512),
    "w0": (36, 256), "b0": (128, 2), "w1p": (128, 512),
    "b1": (128, 2), "w2p": (128, 64), "b2": (32, 1), "rw0": (32, 256),
    "rb0": (128, 2), "rw1p": (128, 64), "rb1p1": (32, 1), "ow0": (32, 256),
    "ob0": (128, 2), "ow1p": (128, 128), "id8a": (8, 128), "id8b": (8, 128),
    "oybcast": (16, 128), "ones1": (1, 128), "neg_iota": (128, 1),
    "sumc": (128, 32),
}

def _fix_excess_waits(nc):
    """This walrus build allows only ONE semaphore wait per instruction.

    For any instruction carrying more, move the extra waits onto fresh NOPs
    inserted immediately before it on the same engine (identical semantics:
    the engine blocks on the same waits at the same program point).
    """
    blocks = list(nc.main_func.blocks)
    for bb in blocks:
        insts = bb.instructions
        i = 0
        while i < len(insts):
            ins = insts[i]
            si = ins.sync_info
            if si is not None and len(si.on_wait) > 1:
                waits = list(si.on_wait)
                extra, keep = waits[:-1], waits[-1:]
                nops = []
                for w in extra:
                    nop = nc.engines[ins.engine].nop(nofuse=True,
                                                     hint="wsplit").ins
                    for obb in blocks:
                        try:
                            obb.instructions.remove(nop)
                            break
                        except ValueError:
                            continue
                    nop.sync_info = mybir.SyncInfo(on_wait=[w], on_update=[])
                    nops.append(nop)
                ins.sync_info = mybir.SyncInfo(on_wait=keep,
                                               on_update=list(si.on_update))
                insts[i:i] = nops
                i += len(nops)
            i += 1


def _rd():
    """dtype for tensors consumed by matmuls (must be f32r-rounded)."""
    return F32R if USE_F32R else F32


def _mm(ap):
    return ap


def _keval(nc, wp, ad, nq, tag, out_bufs=2):
    """K(a) for a = |d| in `ad` ([128, nq] SBUF) -> SBUF tile [128, nq].

    K(a) = P1(min(a,1)) + P2(clamp(a,1,2)), both pieces vanish at the seams:
      P1(t) = ((A+2)t - (A+3)) t^2 + 1
      P2(t) = A(t^3 - 5t^2 + 8t - 4)
    computed as m + n + w with
      m = ((A+2)a1 - (A+3)) * a1^2
      n = (A*a2 - 5A) * a2^2
      w = 8A*a2 + (1 - 4A)
    """
    a1 = wp.tile([128, nq], F32, tag=f"{tag}a1")
    a2 = wp.tile([128, nq], F32, tag=f"{tag}a2")
    s1 = wp.tile([128, nq], F32, tag=f"{tag}s1")
    s2 = wp.tile([128, nq], F32, tag=f"{tag}s2")
    ut = wp.tile([128, nq], F32, tag=f"{tag}u")
    vt = wp.tile([128, nq], F32, tag=f"{tag}v")
    wt = wp.tile([128, nq], F32, tag=f"{tag}w")
    kt = wp.tile([128, nq], _rd(), tag=f"{tag}k", bufs=out_bufs)
    nc.gpsimd.tensor_scalar(a1[:, :], ad[:, :], 1.0, None, ALU.min)
    nc.gpsimd.tensor_scalar(a2[:, :], ad[:, :], 2.0, 1.0, ALU.min, ALU.max)
    nc.gpsimd.tensor_scalar(ut[:, :], a1[:, :], A + 2.0, -(A + 3.0),
                            ALU.mult, ALU.add)
    nc.gpsimd.tensor_scalar(vt[:, :], a2[:, :], A, -5.0 * A, ALU.mult, ALU.add)
    nc.scalar.square(s1[:, :], a1[:, :])
    nc.scalar.square(s2[:, :], a2[:, :])
    nc.gpsimd.tensor_scalar(wt[:, :], a2[:, :], 8.0 * A, 1.0 - 4.0 * A,
                            ALU.mult, ALU.add)
    nc.vector.tensor_mul(ut[:, :], ut[:, :], s1[:, :])
    nc.vector.tensor_mul(vt[:, :], vt[:, :], s2[:, :])
    nc.vector.tensor_add(ut[:, :], ut[:, :], vt[:, :])
    nc.vector.tensor_add(kt[:, :], ut[:, :], wt[:, :])
    return kt


_MM_FED = {"x_loc", "bandx0", "v0all", "w0", "w1p", "w2p", "rw0", "rw1p",
           "ow0", "ow1p", "id8a", "id8b", "oybcast", "ones1", "sumc",
           "relrows"}


def _build_program():
    nc = bass.Bass()
    RD = _rd()
    P = {n: nc.declare_dram_parameter(n, list(s), RD if n in _MM_FED else F32,
                                      isOutput=False)
         for n, s in _SHAPES.items()}
    outp = nc.declare_dram_parameter("outp", [C, OYS * 256], F32, isOutput=True)

    with tile.TileContext(nc) as tc:
        with (
            tc.tile_pool(name="consts", bufs=1) as cp,
            tc.tile_pool(name="work", bufs=2) as wp,
            tc.tile_pool(name="psbig", bufs=2, space="PSUM") as psb,
            tc.tile_pool(name="pssmall", bufs=2, space="PSUM") as pss,
            tc.tile_pool(name="ps256", bufs=4, space="PSUM") as ps2,
            tc.tile_pool(name="psout", bufs=1, space="PSUM") as pso,
        ):
            ct = {}
            for n in _SHAPES:
                if n == "relrows":
                    continue  # streamed from DRAM per chunk
                t = cp.tile(list(_SHAPES[n]), RD if n in _MM_FED else F32,
                            tag=n, name=n + "_sb")
                nc.gpsimd.dma_start(out=t[:, :], in_=P[n][:, :])
                ct[n] = t

            for g in range(N_GROUPS):
                y16 = wp.tile([16, 256], RD, tag="y16", name=f"y16_{g}")
                bandx_tiles = {}
                rpo_tiles = {}
                for cc in range(8):
                    chunk = g * 8 + cc
                    oyA, oyB = 2 * chunk, 2 * chunk + 1

                    # ---------- first sample -> inp
                    inp = wp.tile([36, 512], RD, tag="inp", bufs=3,
                                  name=f"inp_{chunk}")
                    qf_ps = pss.tile([32, 512], F32, tag="pssmall", name="qf_ps")
                    for t_i, oy in ((0, oyA), (1, oyB)):
                        sp = _s_prog(oy)
                        h0 = ps2.tile([128, 256], F32, tag="ps256", name="h0")
                        nc.tensor.matmul(
                            h0[:, :],
                            _mm(ct["x_loc"][:, (sp + 2) * 32:(sp + 6) * 32]),
                            _mm(ct["bandx0"][:, :]),
                            start=True, stop=True)
                        h0s = wp.tile([128, 256], RD, tag="h0s", bufs=2,
                                      name="h0s")
                        nc.scalar.copy(h0s[:, :], h0[:, :])
                        nc.tensor.matmul(
                            qf_ps[:, t_i * 256:(t_i + 1) * 256],
                            _mm(ct["v0all"][:, oy * 32:oy * 32 + 32]),
                            _mm(h0s[:, :]), start=True, stop=True)
                    nc.scalar.copy(inp[0:32, :], qf_ps[:, :])
                    nc.sync.dma_start(
                        out=inp[32:36, :],
                        in_=P["relrows"][:, chunk * 512:(chunk + 1) * 512])

                    # ---------- MLP
                    h1s = []
                    for mh in range(2):
                        ps = psb.tile([128, 512], F32, tag="psbig", name="l1ps")
                        nc.tensor.matmul(ps[:, :],
                                         _mm(ct["w0"][:, mh * 128:(mh + 1) * 128]),
                                         _mm(inp[:, :]), start=True, stop=True)
                        sb = wp.tile([128, 512], RD, tag=f"h1s{mh}", bufs=2,
                                     name=f"h1s{mh}")
                        if mh == 0:
                            nc.scalar.activation(sb[:, :], ps[:, :], AF.Relu,
                                                 bias=ct["b0"][:, 0:1], scale=1.0)
                        else:
                            nc.vector.tensor_scalar(sb[:, :], ps[:, :],
                                                    ct["b0"][:, 1:2], 0.0,
                                                    ALU.add, ALU.max)
                        h1s.append(sb)
                    h2s = []
                    for mh in range(2):
                        ps = psb.tile([128, 512], F32, tag="psbig", name="l2ps")
                        for kh in range(2):
                            nc.tensor.matmul(
                                ps[:, :],
                                _mm(ct["w1p"][:, kh * 256 + mh * 128:kh * 256 + (mh + 1) * 128]),
                                _mm(h1s[kh][:, :]),
                                start=(kh == 0), stop=(kh == 1))
                        sb = wp.tile([128, 512], RD, tag=f"h2s{mh}", bufs=2,
                                     name=f"h2s{mh}")
                        if mh == 0:
                            nc.scalar.activation(sb[:, :], ps[:, :], AF.Relu,
                                                 bias=ct["b1"][:, 0:1], scale=1.0)
                        else:
                            nc.vector.tensor_scalar(sb[:, :], ps[:, :],
                                                    ct["b1"][:, 1:2], 0.0,
                                                    ALU.add, ALU.max)
                        h2s.append(sb)
                    pred_ps = pss.tile([32, 512], F32, tag="pssmall", name="pred_ps")
                    for kh in range(2):
                        nc.tensor.matmul(pred_ps[:, :],
                                         _mm(ct["w2p"][:, kh * 32:(kh + 1) * 32]),
                                         _mm(h2s[kh][:, :]),
                                         start=(kh == 0), stop=(kh == 1))
                    preds = wp.tile([32, 512], RD, tag="preds", bufs=2,
                                    name="preds")
                    nc.scalar.activation(preds[:, :], pred_ps[:, :], AF.Identity,
                                         bias=ct["b2"][:, 0:1], scale=1.0)

                    def head_hidden(wname, bname, tagp, act_both=False):
                        outs = []
                        for mh in range(2):
                            ps = psb.tile([128, 512], F32, tag="psbig",
                                          name=f"{tagp}ps")
                            nc.tensor.matmul(ps[:, :],
                                             _mm(ct[wname][:, mh * 128:(mh + 1) * 128]),
                                             _mm(preds[:, :]),
                                             start=True, stop=True)
                            sb = wp.tile([128, 512], RD, tag=f"{tagp}{mh}",
                                         bufs=2, name=f"{tagp}{mh}")
                            if mh == 0 or act_both:
                                nc.scalar.activation(sb[:, :], ps[:, :], AF.Relu,
                                                     bias=ct[bname][:, mh:mh + 1],
                                                     scale=1.0)
                            else:
                                nc.vector.tensor_scalar(sb[:, :], ps[:, :],
                                                        ct[bname][:, 1:2], 0.0,
                                                        ALU.add, ALU.max)
                            outs.append(sb)
                        return outs

                    rhid = head_hidden("rw0", "rb0", "rh")
                    ohid = head_hidden("ow0", "ob0", "oh")
                    rout_ps = pss.tile([32, 512], F32, tag="pssmall", name="rout_ps")
                    for kh in range(2):
                        nc.tensor.matmul(rout_ps[:, :],
                                         _mm(ct["rw1p"][:, kh * 32:(kh + 1) * 32]),
                                         _mm(rhid[kh][:, :]),
                                         start=(kh == 0), stop=(kh == 1))
                    rpo = wp.tile([32, 512], F32, tag="rpo", bufs=10,
                                  name=f"rpo_{chunk}")
                    nc.scalar.activation(rpo[:, :], rout_ps[:, :], AF.Identity,
                                         bias=ct["rb1p1"][:, 0:1], scale=1.0)
                    rpo_tiles[cc] = rpo
                    off_ps = pss.tile([64, 512], F32, tag="pssmall", name="off_ps")
                    for kh in range(2):
                        nc.tensor.matmul(off_ps[:, :],
                                         _mm(ct["ow1p"][:, kh * 64:(kh + 1) * 64]),
                                         _mm(ohid[kh][:, :]),
                                         start=(kh == 0), stop=(kh == 1))

                    # ---------- offset rows (broadcast only the small
                    # offsets through the PE; position bases stay fp32 in
                    # host tables dstat/candc)
                    offxrow = wp.tile([1, 512], RD, tag="offxrow", bufs=2,
                                      name="offxrow")
                    nc.vector.tensor_copy(offxrow[:, :], off_ps[0:1, :])
                    iy2t = wp.tile([64, 512], RD, tag="iy2t", bufs=2, name="iy2t")
                    for t_i, oy in ((0, oyA), (1, oyB)):
                        nc.vector.tensor_scalar(
                            iy2t[32:33, t_i * 256:(t_i + 1) * 256],
                            off_ps[32:33, t_i * 256:(t_i + 1) * 256],
                            64.0, None, ALU.mult)
                        nc.sync.dma_start(
                            out=y16[2 * cc + t_i:2 * cc + t_i + 1, :],
                            in_=iy2t[32:33, t_i * 256:(t_i + 1) * 256])

                    # ---------- BandX eval: d = dstat - 64*offx
                    bx_ps = psb.tile([128, 512], F32, tag="psbig", name="bx_ps")
                    nc.tensor.matmul(bx_ps[:, :], _mm(ct["ones1"][:, :]),
                                     _mm(offxrow[:, :]), start=True, stop=True)
                    et = wp.tile([128, 512], F32, tag="bxe", name="bxe")
                    nc.vector.scalar_tensor_tensor(
                        et[:, :], bx_ps[:, :], -64.0, ct["dstat"][:, :],
                        ALU.mult, ALU.add)
                    ad0 = wp.tile([128, 512], F32, tag="bxad", name="bxad")
                    nc.scalar.activation(ad0[:, :], et[:, :], AF.Abs)
                    bandx = _keval(nc, wp, ad0, 512, "bx", out_bufs=10)
                    bandx_tiles[cc] = bandx

                # ---------- compact vertical weights for this group
                cwyin_ps = ps2.tile([128, 256], F32, tag="ps256", name="cwyin_ps")
                nc.tensor.matmul(cwyin_ps[:, :], _mm(ct["oybcast"][:, :]),
                                 _mm(y16[:, :]), start=True, stop=True)
                dyt = wp.tile([128, 256], F32, tag="cwdy", name="cwdy")
                nc.vector.tensor_scalar(dyt[:, :], cwyin_ps[:, :],
                                        ct["candc"][:, g:g + 1], None,
                                        ALU.subtract)
                ady = wp.tile([128, 256], F32, tag="cwady", name="cwady")
                nc.scalar.activation(ady[:, :], dyt[:, :], AF.Abs)
                cwyp = _keval(nc, wp, ady, 256, "cw", out_bufs=2)

                # ---------- gather + final per chunk (2 oys packed)
                for cc in range(8):
                    chunk = g * 8 + cc
                    oyA, oyB = 2 * chunk, 2 * chunk + 1
                    oA, oB = oyA % 16, oyB % 16
                    spA, spB = _s_prog(oyA), _s_prog(oyB)
                    cwo2 = wp.tile([8, 512], RD, tag="cwo", bufs=2,
                                   name="cwo2")
                    nc.sync.dma_start(out=cwo2[:, 0:256],
                                      in_=cwyp[oA * 8:oA * 8 + 8, :])
                    nc.sync.dma_start(out=cwo2[:, 256:512],
                                      in_=cwyp[oB * 8:oB * 8 + 8, :])
                    hgps = []
                    for half in range(2):
                        ps = ps2.tile([128, 512], F32, tag="ps256",
                                      name="hgps")
                        for t_i, sp in ((0, spA), (1, spB)):
                            nc.tensor.matmul(
                                ps[:, t_i * 256:(t_i + 1) * 256],
                                _mm(ct["x_loc"][:, sp * 32 + half * 128:
                                                sp * 32 + half * 128 + 128]),
                                _mm(bandx_tiles[cc][:, t_i * 256:(t_i + 1) * 256]),
                                start=True, stop=True)
                        hgps.append(ps)
                    out0 = pss.tile([32, 512], F32, tag="pssmall", name="out0")
                    for half in range(2):
                        wyps = ps2.tile([128, 512], F32, tag="ps256",
                                        name="wyps")
                        nc.tensor.matmul(
                            wyps[:, :],
                            _mm(ct["id8a" if half == 0 else "id8b"][:, :]),
                            _mm(cwo2[:, :]), start=True, stop=True)
                        wys = wp.tile([128, 512], F32, tag="wys", bufs=2,
                                      name="wys")
                        nc.scalar.copy(wys[:, :], wyps[:, :])
                        hwt = wp.tile([128, 512], RD, tag=f"hw{half}",
                                      bufs=2, name=f"hw{half}")
                        nc.vector.tensor_mul(hwt[:, :], hgps[half][:, :],
                                             wys[:, :])
                        nc.tensor.matmul(
                            out0[:, :], _mm(ct["sumc"][:, :]), _mm(hwt[:, :]),
                            start=(half == 0), stop=(half == 1))
                    outt = wp.tile([32, 512], F32, tag="outt", bufs=2,
                                   name="outt")
                    nc.vector.tensor_mul(outt[:, :], out0[:, :],
                                         rpo_tiles[cc][:, :])
                    nc.sync.dma_start(
                        out=outp[:, oyA * 256:oyA * 256 + 512],
                        in_=outt[:, :])

    _fix_excess_waits(nc)
    return nc


_PROGRAM = None
_LAST_EXEC_NS = None


def kernel(**inputs):
    global _PROGRAM
    if _PROGRAM is None:
        _PROGRAM = _build_program()
    nc = _PROGRAM
    shared = _build_shared(inputs)
    in_maps = []
    for core in range(N_CORES):
        b, h = divmod(core, 2)
        in_maps.append(_build_core_inputs(inputs, b, h, shared))
    trace = os.environ.get("KTRACE", "0") == "1"
    try:
        res = run_bass_kernel_spmd(nc, in_maps, list(range(N_CORES)),
                                   trace=trace)
    except Exception:
        if not trace:
            raise
        res = run_bass_kernel_spmd(nc, in_maps, list(range(N_CORES)))
    global _LAST_EXEC_NS
    _LAST_EXEC_NS = res.exec_time_ns
    out = np.zeros((B, C, HO, WO), np.float32)
    for core in range(N_CORES):
        b, h = divmod(core, 2)
        o = res.results[core]["outp"].reshape(C, OYS, 256)
        out[b, :, h * 128:h * 128 + 128, :] = o
    return out



# revision 40
# speedup vs baseline: 1.0103x; 1.0103x over previous
"""Trainium2 Bass kernel for nn_CRM_14886356648008 (LIIF-style SR module).

Sharding: 8 cores = 4 images x 2 output-row halves. Each core computes
out[b, :, h*128:(h+1)*128, :] from the full input image plus static tables.

v3 design (vs the f32r baseline at ~850us):
  - First (regular-grid) bicubic sample as fp8 DoubleRow matmuls with the
    vertical taps fused into static per-parity weights (the MLP's *feature*
    input tolerates fp8; the offset path does not care about it).
  - MLP trunk + offset head in fp16 (the predicted sample offset needs
    ~1% accuracy because d(out)/d(offset) reaches ~4 per pixel); routing
    head output in fp8 with a DoubleRow head matmul (routing tolerates it).
  - Per-query biases ride inside matmuls as spare contraction rows carrying
    ones, so PSUM->SBUF conversions are single fused activation ops.
  - Bicubic weights evaluated exactly via
        K/1.25 = min(sa + (0.8 - 1.8 s), (-0.6 a + 0.6)(min(a,2)-2)^2),
    s = a^2, sa = s*a, in fp16 on [128, 1024] chunk-pair tiles; the 1.25
    is folded into the gather's static operands. Static distance tables are
    pre-clipped so fp16 never overflows (clipped taps evaluate to exactly 0).
  - Vertical-weight broadcast via per-(oy,half) one-hot selector matmuls.
  - Group-level software pipelining: group g's MLP work is interleaved with
    group g-1's gather at chunk granularity so the in-order engine streams
    never drain at phase boundaries.
"""
import os
import numpy as np
import ml_dtypes as md

import concourse.bass as bass
import concourse.tile as tile
from concourse import mybir
from concourse.bass_utils import run_bass_kernel_spmd

F32 = mybir.dt.float32
F32R = mybir.dt.float32r
FP16 = mybir.dt.float16
FP8 = mybir.dt.float8e4
AF = mybir.ActivationFunctionType
ALU = mybir.AluOpType
DR = mybir.MatmulPerfMode.DoubleRow

A = -0.75
B, C, H, W = 4, 32, 128, 128
SCALE = 2
HO, WO = H * SCALE, W * SCALE
J = 76        # x_loc free rows per channel
SHIFT = 7     # x_loc[j] = image row j + 64*h - SHIFT
N_CORES = 8
OYS = 128     # output rows per core
N_GROUPS = 8  # 16 oys per group

JC = J * C
RHS = 512.0   # fp8 storage scale for the routing hidden


def _q8(x):
    return np.ascontiguousarray(np.asarray(x, np.float32)).astype(md.float8_e4m3fn)


def _q16(x):
    return np.ascontiguousarray(np.asarray(x, np.float32)).astype(np.float16)


def _f32(x):
    return np.ascontiguousarray(np.asarray(x, np.float32))


# ----------------------------------------------------------------------------
# host-side reference math (for the static rel-coord tables)
# ----------------------------------------------------------------------------

def _cubic_weights(t):
    x = t + 1.0
    w0 = ((A * x - 5.0 * A) * x + 8.0 * A) * x - 4.0 * A
    w1 = ((A + 2.0) * t - (A + 3.0)) * t * t + 1.0
    s = 1.0 - t
    w2 = ((A + 2.0) * s - (A + 3.0)) * s * s + 1.0
    w3 = 1.0 - w0 - w1 - w2
    return np.stack([w0, w1, w2, w3], axis=-1)


def _grid_sample_bicubic_np(feat, gx, gy):
    Bn, Cn, Hn, Wn = feat.shape
    ix = ((gx + 1.0) * Wn - 1.0) * 0.5
    iy = ((gy + 1.0) * Hn - 1.0) * 0.5
    ix0 = np.floor(ix)
    iy0 = np.floor(iy)
    wx = _cubic_weights(ix - ix0)
    wy = _cubic_weights(iy - iy0)
    ix0 = ix0.astype(np.int32)
    iy0 = iy0.astype(np.int32)
    ff = feat.reshape(Bn, Cn, Hn * Wn)
    out = np.zeros((Bn, Cn, gx.shape[1]), feat.dtype)
    for i in range(4):
        yi = iy0 - 1 + i
        yok = (yi >= 0) & (yi < Hn)
        yc = np.clip(yi, 0, Hn - 1)
        for jj in range(4):
            xj = ix0 - 1 + jj
            ok = yok & (xj >= 0) & (xj < Wn)
            xc = np.clip(xj, 0, Wn - 1)
            v = np.take_along_axis(ff, (yc * Wn + xc)[:, None, :], axis=2)
            w = wy[..., i] * wx[..., jj] * ok
            out = out + v * w[:, None, :].astype(feat.dtype)
    return out


def _make_coord(Hn, Wn):
    y = -1.0 + (2.0 * np.arange(Hn, dtype=np.float32) + 1.0) / Hn
    x = -1.0 + (2.0 * np.arange(Wn, dtype=np.float32) + 1.0) / Wn
    yy, xx = np.meshgrid(y, x, indexing="ij")
    return np.stack([yy, xx], axis=-1).astype(np.float32)


def _rel_table():
    """rel features [Q, 4] = [rel_coord_y, rel_coord_x, rel_cell_y, rel_cell_x]."""
    Q = HO * WO
    coord = _make_coord(HO, WO).reshape(Q, 2)
    cell = np.ones((Q, 2), np.float32)
    cell[0] *= 2.0 / HO
    cell[1] *= 2.0 / WO
    cc = np.clip(coord, -1.0 + 1e-6, 1.0 - 1e-6)
    gy, gx = cc[None, :, 0], cc[None, :, 1]
    fc = np.broadcast_to(_make_coord(H, W).transpose(2, 0, 1)[None], (1, 2, H, W))
    q_coord = _grid_sample_bicubic_np(fc, gx, gy).transpose(0, 2, 1)[0]
    hw = np.array([H, W], np.float32)
    rel_coord = (coord - q_coord) * hw
    rel_cell = cell * hw
    return np.concatenate([rel_coord, rel_cell], axis=-1).astype(np.float32)


def _ky_l(oy_l):
    return int(np.floor(oy_l / 2.0 - 0.25))


def _s_prog(oy_l):
    return _ky_l(oy_l) + 4


# ----------------------------------------------------------------------------
# static tables (shared across cores)
# ----------------------------------------------------------------------------

def _build_shared(inputs):
    rel = _rel_table()

    w0 = _f32(inputs["mlp_w0"])
    b0 = _f32(inputs["mlp_b0"])
    w1 = _f32(inputs["mlp_w1"])
    b1 = _f32(inputs["mlp_b1"])
    w2 = _f32(inputs["mlp_w2"])
    b2 = _f32(inputs["mlp_b2"])
    rw0 = _f32(inputs["rout_w0"])
    rb0 = _f32(inputs["rout_b0"])
    rw1 = _f32(inputs["rout_w1"])
    rb1 = _f32(inputs["rout_b1"])
    ow0 = _f32(inputs["off_w0"])
    ob0 = _f32(inputs["off_b0"])
    ow1 = _f32(inputs["off_w1"])
    b_off = _f32(inputs["off_b1"])

    bandx0 = np.zeros((W, WO), np.float32)
    for ox in range(WO):
        ix = ox / 2.0 - 0.25
        kx = int(np.floor(ix))
        wts = _cubic_weights(np.float32(ix - kx))
        for tap in range(4):
            wc = kx - 1 + tap
            if 0 <= wc < W:
                bandx0[wc, ox] = wts[tap]
    # first sample fused vertical x horizontal: bandq8[w, par, rp, r', ox]
    # = wy0[par][2rp+r'] * bandx0[w, ox]; DR moving operand with x_loc8f
    # j-slices stationary.
    bandq8 = np.zeros((W, 2, 2, 2, WO), np.float32)
    for par, t in ((0, 0.75), (1, 0.25)):
        wy0 = _cubic_weights(np.float32(t))
        for rp in range(2):
            for r2 in range(2):
                bandq8[:, par, rp, r2, :] = wy0[2 * rp + r2] * bandx0

    # l1: K = 37 (32 qf + 4 rel + ones row)
    w0b = np.zeros((37, 256), np.float32)
    w0b[0:36] = w0
    w0b[36] = b0

    # l2: K = 2x128 fp16 (kh-sliced)
    w1p = np.zeros((128, 2, 256), np.float32)
    for kh in range(2):
        w1p[:, kh, :] = w1[kh * 128:(kh + 1) * 128, :]

    w2p = np.zeros((128, 2, 32), np.float32)
    for kh in range(2):
        w2p[:, kh, :] = w2[kh * 128:(kh + 1) * 128, :]

    # heads hidden: K = 33 (row 32 = ones -> bias)
    rw0b = np.zeros((33, 256), np.float32)
    rw0b[0:32] = rw0
    rw0b[32] = rb0
    ow0b = np.zeros((33, 256), np.float32)
    ow0b[0:32] = ow0
    ow0b[32] = ob0

    # routing out: DR over fp8 rhid stored at 512x -> rout_ps = 4096*routlin
    rw1dr = np.zeros((128, 2, 32), np.float32)
    for kt in range(2):
        rw1dr[:, kt, :] = 8.0 * rw1[kt * 128:(kt + 1) * 128, :]
    # offset out: fp16, x -> partition 0, y -> partition 32
    ow1p = np.zeros((128, 2, 64), np.float32)
    for kh in range(2):
        ow1p[:, kh, 0] = ow1[kh * 128:(kh + 1) * 128, 0]
        ow1p[:, kh, 32] = ow1[kh * 128:(kh + 1) * 128, 1]

    # static distance table for the offset horizontal band (b_off folded),
    # clipped to keep the fp16 eval in range (clipped taps evaluate to 0)
    ox = np.arange(256, dtype=np.float32)
    ixb = ox / 2.0 - 0.25 + 64.0 * b_off[0]
    dcol = np.clip(np.arange(W, dtype=np.float32)[:, None] - ixb[None, :], -8.0, 8.0)
    dstatp = np.tile(dcol, (1, 4))        # pair level [128, 1024]

    # vertical-weight selector; 1.25 compensates the Ky eval's 1/1.25
    sel = np.zeros((128, 32 * 128), np.float32)
    for half in range(2):
        for o in range(16):
            Sm = np.zeros((128, 128), np.float32)
            for m in range(128):
                r2 = half * 4 + m // 32
                Sm[o * 8 + r2, m] = 1.25
            sel[:, (half * 16 + o) * 128:(half * 16 + o + 1) * 128] = Sm

    sumc = np.zeros((128, 32), np.float32)
    sumc[np.arange(128), np.arange(128) % 32] = 1.0

    oybcast = np.zeros((16, 128), np.float32)
    oybcast[np.arange(128) // 8, np.arange(128)] = 1.0

    static = {
        "bandq8": _q8(bandq8.reshape(128, 2048)),
        "w0b": _q16(w0b),
        "w1p": _q16(w1p.reshape(128, 512)),
        "w2p": _q16(w2p.reshape(128, 64)),
        "rw0b": _q16(rw0b),
        "ow0b": _q16(ow0b),
        "rw1dr": _q8(rw1dr.reshape(128, 64)),
        "ow1p": _q16(ow1p.reshape(128, 128)),
        "b1a": _f32(b1[0:128].reshape(128, 1)),
        "b1b": _f32(b1[128:256].reshape(128, 1)),
        "b2c": _f32(b2.reshape(32, 1)),
        "rb1p1": _f32((1.0 + rb1).reshape(32, 1)),
        "dstatp": _q16(dstatp),
        "sel": _q16(sel),
        "sumc16": _q16(sumc),
        "oybc64": _f32(64.0 * oybcast),
        "onesm": np.full((1, 128), -64.0, np.float32),
    }
    return {"rel": rel, "b_off": b_off, "static": static}


# ----------------------------------------------------------------------------
# per-core input tables
# ----------------------------------------------------------------------------

def _build_core_inputs(inputs, b, h, shared):
    x = np.asarray(inputs["x"])[b]
    x_loc = np.zeros((W, J, C), np.float32)
    rows = np.arange(J) + 64 * h - SHIFT
    valid = (rows >= 0) & (rows < H)
    x_loc[:, valid, :] = x[:, rows[valid], :].transpose(2, 1, 0)
    x_loc = x_loc.reshape(W, JC)

    b_off = shared["b_off"]
    # candcn[(o,r), g] = -(candpos - iy(oy) - 64*b_off_y); Act-bias form so
    # dy = 64*offy + candcn in one op (K is even in d; sentinel 16 -> K = 0)
    candcn = np.zeros((128, 8), np.float32)
    for g in range(8):
        for o in range(16):
            oy_g = h * 128 + 16 * g + o
            iy = oy_g / 2.0 - 0.25
            ky = int(np.floor(iy))
            for r in range(8):
                row = ky - 3 + r
                candcn[o * 8 + r, g] = -(
                    row - iy - 64.0 * b_off[1] if 0 <= row < H else 16.0)

    q0 = h * 128 * 256
    # kt rows 32..36 of the l1 input: 4 rel features + the bias-ones row
    relrows = np.zeros((5, OYS * 256), np.float32)
    relrows[0:4] = shared["rel"][q0:q0 + OYS * 256].T
    relrows[4] = 1.0

    d = {
        # 1.25 compensates the Kx eval's 1/1.25 scaling (see _keval)
        "x_loc16": _q16(1.25 * x_loc),
        "x_loc8f": _q8(x_loc),
        "candcn": _f32(candcn),
        "relrows16": _q16(relrows),
    }
    d.update(shared["static"])
    return d


_SHAPES = {
    "x_loc16": ((W, JC), FP16),
    "x_loc8f": ((W, JC), FP8),
    "candcn": ((128, 8), F32),
    "relrows16": ((5, OYS * 256), FP16),
    "bandq8": ((128, 2048), FP8),
    "w0b": ((37, 256), FP16),
    "w1p": ((128, 512), FP16),
    "w2p": ((128, 64), FP16),
    "rw0b": ((33, 256), FP16),
    "ow0b": ((33, 256), FP16),
    "rw1dr": ((128, 64), FP8),
    "ow1p": ((128, 128), FP16),
    "b1a": ((128, 1), F32),
    "b1b": ((128, 1), F32),
    "b2c": ((32, 1), F32),
    "rb1p1": ((32, 1), F32),
    "dstatp": ((128, 1024), FP16),
    "sel": ((128, 32 * 128), FP16),
    "sumc16": ((128, 32), FP16),
    "oybc64": ((16, 128), F32R),
    "onesm": ((1, 128), F32R),
}

# tiles reshaped to >2D on-chip
_TILE3D = {
    "x_loc8f": (W, J, C),
    "bandq8": (W, 2, 2, 2, 256),
    "w1p": (128, 2, 256),
    "w2p": (128, 2, 32),
    "rw1dr": (128, 2, 32),
    "ow1p": (128, 2, 64),
}


def _fix_excess_waits(nc):
    """This walrus build allows only ONE semaphore wait per instruction.

    For any instruction carrying more, move the extra waits onto fresh NOPs
    inserted immediately before it on the same engine (identical semantics:
    the engine blocks on the same waits at the same program point).
    """
    blocks = list(nc.main_func.blocks)
    for bb in blocks:
        insts = bb.instructions
        i = 0
        while i < len(insts):
            ins = insts[i]
            si = ins.sync_info
            if si is not None and len(si.on_wait) > 1:
                waits = list(si.on_wait)
                extra, keep = waits[:-1], waits[-1:]
                nops = []
                for w in extra:
                    nop = nc.engines[ins.engine].nop(nofuse=True,
                                                     hint="wsplit").ins
                    for obb in blocks:
                        try:
                            obb.instructions.remove(nop)
                            break
                        except ValueError:
                            continue
                    nop.sync_info = mybir.SyncInfo(on_wait=[w], on_update=[])
                    nops.append(nop)
                ins.sync_info = mybir.SyncInfo(on_wait=keep,
                                               on_update=list(si.on_update))
                insts[i:i] = nops
                i += len(nops)
            i += 1


def _keval(nc, wp, et, nq, tag, out_bufs=2):
    """Exact bicubic kernel K(|et|)/1.25 -> fp16 tile [128, nq].

    K/1.25 = min(sa + (0.8 - 1.8 s), (-0.6 a + 0.6)(min(a,2)-2)^2), s = a^2,
    sa = s*a. Consumers' static weights carry the 1.25 back. Valid for the
    pre-clipped |et| <= ~16 range (fp16-safe).
    """
    a = wp.tile([128, nq], FP16, tag=f"{tag}a", bufs=3)
    s = wp.tile([128, nq], FP16, tag=f"{tag}s", bufs=3)
    c1 = wp.tile([128, nq], FP16, tag=f"{tag}c1", bufs=3)
    sa = wp.tile([128, nq], FP16, tag=f"{tag}sa", bufs=3)
    p1 = wp.tile([128, nq], FP16, tag=f"{tag}p1", bufs=3)
    t2 = wp.tile([128, nq], FP16, tag=f"{tag}t2", bufs=3)
    q2 = wp.tile([128, nq], FP16, tag=f"{tag}q2", bufs=3)
    r1 = wp.tile([128, nq], FP16, tag=f"{tag}r1", bufs=3)
    m2 = wp.tile([128, nq], FP16, tag=f"{tag}m2", bufs=3)
    kt = wp.tile([128, nq], FP16, tag=f"{tag}kt", bufs=out_bufs)
    nc.scalar.activation(a[:, :], et[:, :], AF.Abs)
    nc.vector.tensor_tensor(s[:, :], et[:, :], et[:, :], ALU.mult)
    nc.gpsimd.tensor_scalar(c1[:, :], s[:, :], -1.8, 0.8, ALU.mult, ALU.add)
    nc.vector.tensor_tensor(sa[:, :], s[:, :], a[:, :], ALU.mult)
    nc.vector.tensor_tensor(p1[:, :], sa[:, :], c1[:, :], ALU.add)
    nc.gpsimd.tensor_scalar(t2[:, :], a[:, :], -2.0, 0.0, ALU.add, ALU.min)
    nc.vector.tensor_tensor(q2[:, :], t2[:, :], t2[:, :], ALU.mult)
    nc.gpsimd.tensor_scalar(r1[:, :], a[:, :], -0.6, 0.6, ALU.mult, ALU.add)
    nc.vector.tensor_tensor(m2[:, :], r1[:, :], q2[:, :], ALU.mult)
    nc.vector.tensor_tensor(kt[:, :], p1[:, :], m2[:, :], ALU.min)
    return kt


def _build_program():
    nc = bass.Bass()
    P = {n: nc.declare_dram_parameter(n, list(s), d, isOutput=False)
         for n, (s, d) in _SHAPES.items()}
    outp = nc.declare_dram_parameter("outp", [C, OYS * 256], F32, isOutput=True)

    with tile.TileContext(nc) as tc:
        with (
            tc.tile_pool(name="consts", bufs=1) as cp,
            tc.tile_pool(name="work", bufs=2) as wp,
            tc.tile_pool(name="psM", bufs=2, space="PSUM") as psM,
            tc.tile_pool(name="psQ", bufs=2, space="PSUM") as psQ,
            tc.tile_pool(name="psG", bufs=2, space="PSUM") as psG,
        ):
            ct = {}
            for n, (s, d) in _SHAPES.items():
                if n == "relrows16":
                    continue  # streamed from DRAM per chunk
                shape = list(_TILE3D.get(n, s))
                t = cp.tile(shape, d, tag=n, name=n + "_sb")
                full = tuple(slice(None) for _ in shape)
                nc.gpsimd.dma_start(out=t[full], in_=P[n][:, :])
                ct[n] = t

            state = {}
            # persistent preds buffers (manual rotation): the bias-ones row
            # (partition 32) is written once and must survive reuse, which a
            # rotating pool-tile generation would flag as a stale read
            preds_bufs = []
            for i in range(3):
                t = cp.tile([33, 512], FP16, tag=f"preds{i}",
                            name=f"preds{i}")
                nc.gpsimd.memset(t[32:33, :], 1.0)
                preds_bufs.append(t)

            def chunk_mlp(g, cc):
                ch = g * 8 + cc
                sub = cc & 1
                oyA, oyB = 2 * ch, 2 * ch + 1
                y16 = state["y16"]

                # ---------- first sample: fused vertical x horizontal DR
                qf_ps = psQ.tile([32, 512], F32, tag="psQ", name="qf_ps")
                for t_i, oy in ((0, oyA), (1, oyB)):
                    sp = _s_prog(oy)
                    par = oy & 1
                    for rp in range(2):
                        j0 = sp + 2 + 2 * rp
                        nc.tensor.matmul(
                            qf_ps[:, t_i * 256:(t_i + 1) * 256],
                            ct["x_loc8f"][:, j0:j0 + 2, :],
                            ct["bandq8"][:, par, rp, :, :],
                            start=(rp == 0), stop=(rp == 1), perf_mode=DR)

                inp = wp.tile([37, 512], FP16, tag="inp", bufs=4,
                              name=f"inp_{ch}")
                nc.scalar.copy(inp[0:32, :], qf_ps[:, :])
                nc.sync.dma_start(
                    out=inp[32:37, :],
                    in_=P["relrows16"][:, ch * 512:(ch + 1) * 512])

                # ---------- MLP trunk (fp16)
                l1ps = psM.tile([128, 1024], F32, tag="psM", name="l1ps")
                for mh in range(2):
                    nc.tensor.matmul(l1ps[:, mh * 512:(mh + 1) * 512],
                                     ct["w0b"][:, mh * 128:(mh + 1) * 128],
                                     inp[:, :], start=True, stop=True)
                h1 = wp.tile([128, 2, 512], FP16, tag="h1", bufs=3, name="h1")
                nc.scalar.activation(h1[:, 0, :], l1ps[:, 0:512], AF.Relu)
                nc.vector.tensor_scalar(h1[:, 1, :], l1ps[:, 512:1024], 0.0,
                                        None, ALU.max)

                l2ps = psM.tile([128, 1024], F32, tag="psM", name="l2ps")
                for mh in range(2):
                    for kh in range(2):
                        nc.tensor.matmul(
                            l2ps[:, mh * 512:(mh + 1) * 512],
                            ct["w1p"][:, kh, mh * 128:(mh + 1) * 128],
                            h1[:, kh, :], start=(kh == 0), stop=(kh == 1))
                h2 = wp.tile([128, 2, 512], FP16, tag="h2", bufs=3, name="h2")
                nc.scalar.activation(h2[:, 0, :], l2ps[:, 0:512], AF.Relu,
                                     bias=ct["b1a"][:, 0:1])
                nc.vector.tensor_scalar(h2[:, 1, :], l2ps[:, 512:1024],
                                        ct["b1b"][:, 0:1], 0.0,
                                        ALU.add, ALU.max)

                pred_ps = psQ.tile([32, 512], F32, tag="psQ", name="pred_ps")
                for kh in range(2):
                    nc.tensor.matmul(pred_ps[:, :], ct["w2p"][:, kh, :],
                                     h2[:, kh, :],
                                     start=(kh == 0), stop=(kh == 1))
                preds = preds_bufs[ch % 3]
                nc.scalar.activation(preds[0:32, :], pred_ps[:, :],
                                     AF.Identity, bias=ct["b2c"][:, 0:1])

                # ---------- heads
                rhps = psM.tile([128, 1024], F32, tag="psM", name="rhps")
                for mh in range(2):
                    nc.tensor.matmul(rhps[:, mh * 512:(mh + 1) * 512],
                                     ct["rw0b"][:, mh * 128:(mh + 1) * 128],
                                     preds[:, :], start=True, stop=True)
                rhid = wp.tile([128, 2, 512], FP8, tag="rhid", bufs=3, name="rhid")
                nc.scalar.activation(rhid[:, 0, :], rhps[:, 0:512], AF.Relu,
                                     scale=RHS)
                nc.vector.tensor_scalar(rhid[:, 1, :], rhps[:, 512:1024],
                                        RHS, 0.0, ALU.mult, ALU.max)

                ohps = psM.tile([128, 1024], F32, tag="psM", name="ohps")
                for mh in range(2):
                    nc.tensor.matmul(ohps[:, mh * 512:(mh + 1) * 512],
                                     ct["ow0b"][:, mh * 128:(mh + 1) * 128],
                                     preds[:, :], start=True, stop=True)
                ohid = wp.tile([128, 2, 512], FP16, tag="ohid", bufs=3, name="ohid")
                nc.scalar.activation(ohid[:, 0, :], ohps[:, 0:512], AF.Relu)
                nc.vector.tensor_scalar(ohid[:, 1, :], ohps[:, 512:1024], 0.0,
                                        None, ALU.max)

                rout_ps = psQ.tile([32, 512], F32, tag="psQ", name="rout_ps")
                nc.tensor.matmul(rout_ps[:, :], ct["rw1dr"][:, :, :],
                                 rhid[:, :, :], start=True, stop=True,
                                 perf_mode=DR)
                rpo = wp.tile([32, 512], FP16, tag="rpo", bufs=20,
                              name=f"rpo_{ch}")
                nc.scalar.activation(rpo[:, :], rout_ps[:, :], AF.Identity,
                                     bias=ct["rb1p1"][:, 0:1],
                                     scale=1.0 / 4096.0)
                state["rpo"][ch] = rpo

                off_ps = psQ.tile([64, 512], F32, tag="psQ", name="off_ps")
                for kh in range(2):
                    nc.tensor.matmul(off_ps[:, :], ct["ow1p"][:, kh, :],
                                     ohid[:, kh, :],
                                     start=(kh == 0), stop=(kh == 1))
                # raw offsets out of PSUM in one op (rows 0 = x, 32 = y)
                oxy = wp.tile([33, 512], F32R, tag="oxy", bufs=4, name="oxy")
                nc.scalar.copy(oxy[:, :], off_ps[0:33, :])
                nc.sync.dma_start(out=y16[2 * cc:2 * cc + 2, :],
                                  in_=oxy[32:33, :])

                # ---------- offset horizontal band (eval at chunk-pair level)
                bx_ps = psQ.tile([128, 512], F32, tag="psQ", name="bx_ps")
                nc.tensor.matmul(bx_ps[:, :], ct["onesm"][:, :], oxy[0:1, :],
                                 start=True, stop=True)
                if sub == 0:
                    state["et"] = wp.tile([128, 1024], FP16, tag="bxet",
                                          bufs=3, name="bxet")
                nc.vector.tensor_tensor(
                    state["et"][:, sub * 512:(sub + 1) * 512], bx_ps[:, :],
                    ct["dstatp"][:, 0:512], ALU.add)
                if sub == 1:
                    state["ktp"][ch // 2] = _keval(nc, wp, state["et"], 1024,
                                                   "bx", out_bufs=10)

            def group_cwy(g):
                y16 = state["y16"]
                cwyin = psQ.tile([128, 256], F32, tag="psQ", name="cwyin")
                nc.tensor.matmul(cwyin[:, :], ct["oybc64"][:, :], y16[:, :],
                                 start=True, stop=True)
                dy = wp.tile([128, 256], FP16, tag="cwdy", name="cwdy")
                nc.scalar.activation(dy[:, :], cwyin[:, :], AF.Identity,
                                     bias=ct["candcn"][:, g:g + 1])
                state["cwyp"][g] = _keval(nc, wp, dy, 256, "cw", out_bufs=2)

            def chunk_gather(g, cc):
                ch = g * 8 + cc
                sub = cc & 1
                oyA, oyB = 2 * ch, 2 * ch + 1
                spA, spB = _s_prog(oyA), _s_prog(oyB)
                ktp = state["ktp"][ch // 2]
                cwyp = state["cwyp"][g]

                out0 = psQ.tile([32, 512], F32, tag="psQ", name="out0")
                for half in range(2):
                    hgps = psG.tile([128, 512], F32, tag="psG", name="hgps")
                    for t_i, sp in ((0, spA), (1, spB)):
                        nc.tensor.matmul(
                            hgps[:, t_i * 256:t_i * 256 + 256],
                            ct["x_loc16"][:, sp * 32 + half * 128:
                                          sp * 32 + half * 128 + 128],
                            ktp[:, sub * 512 + t_i * 256:
                                sub * 512 + t_i * 256 + 256],
                            start=True, stop=True)
                    wyps = psG.tile([128, 512], F32, tag="psG", name="wyps")
                    for t_i in range(2):
                        o = (2 * cc + t_i) % 16
                        si = (half * 16 + o) * 128
                        nc.tensor.matmul(
                            wyps[:, t_i * 256:t_i * 256 + 256],
                            ct["sel"][:, si:si + 128],
                            cwyp[:, :], start=True, stop=True)
                    wys = wp.tile([128, 512], FP16, tag="wys", bufs=4,
                                  name="wys")
                    if half == 0:
                        nc.scalar.copy(wys[:, :], wyps[:, :])
                    else:
                        nc.vector.tensor_scalar(wys[:, :], wyps[:, :], 1.0,
                                                None, ALU.mult)
                    hwt = wp.tile([128, 512], FP16, tag="hwt", bufs=4,
                                  name="hwt")
                    nc.vector.tensor_tensor(hwt[:, :], hgps[:, :], wys[:, :],
                                            ALU.mult)
                    nc.tensor.matmul(
                        out0[:, :], ct["sumc16"][:, :], hwt[:, :],
                        start=(half == 0), stop=(half == 1))
                if sub == 0:
                    state["outtp"] = wp.tile([32, 1024], F32, tag="outtp",
                                             name="outtp")
                nc.vector.tensor_tensor(
                    state["outtp"][:, sub * 512:(sub + 1) * 512], out0[:, :],
                    state["rpo"][ch][:, :], ALU.mult)
                if sub == 1:
                    nc.sync.dma_start(
                        out=outp[:, (ch - 1) * 512:(ch + 1) * 512],
                        in_=state["outtp"][:, :])

            # software pipeline: group g's MLP interleaves with group g-1's
            # gather at chunk granularity
            state["rpo"] = {}
            state["ktp"] = {}
            state["cwyp"] = {}
            for g in range(N_GROUPS):
                state["y16"] = wp.tile([16, 256], F32R, tag="y16",
                                       name=f"y16_{g}")
                for cc in range(8):
                    chunk_mlp(g, cc)
                    if g > 0:
                        chunk_gather(g - 1, cc)
                group_cwy(g)
            for cc in range(8):
                chunk_gather(N_GROUPS - 1, cc)

    _fix_excess_waits(nc)
    return nc


_PROGRAM = None
_LAST_EXEC_NS = None


def kernel(**inputs):
    global _PROGRAM
    if _PROGRAM is None:
        _PROGRAM = _build_program()
    nc = _PROGRAM
    shared = _build_shared(inputs)
    in_maps = []
    for core in range(N_CORES):
        b, h = divmod(core, 2)
        in_maps.append(_build_core_inputs(inputs, b, h, shared))
    trace = os.environ.get("KTRACE", "0") == "1"
    try:
        res = run_bass_kernel_spmd(nc, in_maps, list(range(N_CORES)),
                                   trace=trace)
    except Exception:
        if not trace:
            raise
        res = run_bass_kernel_spmd(nc, in_maps, list(range(N_CORES)))
    global _LAST_EXEC_NS
    _LAST_EXEC_NS = res.exec_time_ns
    out = np.zeros((B, C, HO, WO), np.float32)
    for core in range(N_CORES):
        b, h = divmod(core, 2)
        o = res.results[core]["outp"].reshape(C, OYS, 256)
        out[b, :, h * 128:h * 128 + 128, :] = o
    return out


# revision 42
# speedup vs baseline: 1.0111x; 1.0008x over previous
"""Trainium2 Bass kernel for nn_CRM_14886356648008 (LIIF-style SR module).

Sharding: 8 cores = 4 images x 2 output-row halves. Each core computes
out[b, :, h*128:(h+1)*128, :] from the full input image plus static tables.

v3 design (vs the f32r baseline at ~850us):
  - First (regular-grid) bicubic sample as fp8 DoubleRow matmuls with the
    vertical taps fused into static per-parity weights (the MLP's *feature*
    input tolerates fp8; the offset path does not care about it).
  - MLP trunk + offset head in fp16 (the predicted sample offset needs
    ~1% accuracy because d(out)/d(offset) reaches ~4 per pixel); routing
    head output in fp8 with a DoubleRow head matmul (routing tolerates it).
  - Per-query biases ride inside matmuls as spare contraction rows carrying
    ones, so PSUM->SBUF conversions are single fused activation ops.
  - Bicubic weights evaluated exactly via
        K/1.25 = min(sa + (0.8 - 1.8 s), (-0.6 a + 0.6)(min(a,2)-2)^2),
    s = a^2, sa = s*a, in fp16 on [128, 1024] chunk-pair tiles; the 1.25
    is folded into the gather's static operands. Static distance tables are
    pre-clipped so fp16 never overflows (clipped taps evaluate to exactly 0).
  - Vertical-weight broadcast via per-(oy,half) one-hot selector matmuls.
  - Group-level software pipelining: group g's MLP work is interleaved with
    group g-1's gather at chunk granularity so the in-order engine streams
    never drain at phase boundaries.
"""
import os
import numpy as np
import ml_dtypes as md

import concourse.bass as bass
import concourse.tile as tile
from concourse import mybir
from concourse.bass_utils import run_bass_kernel_spmd

F32 = mybir.dt.float32
F32R = mybir.dt.float32r
FP16 = mybir.dt.float16
FP8 = mybir.dt.float8e4
AF = mybir.ActivationFunctionType
ALU = mybir.AluOpType
DR = mybir.MatmulPerfMode.DoubleRow

A = -0.75
B, C, H, W = 4, 32, 128, 128
SCALE = 2
HO, WO = H * SCALE, W * SCALE
J = 76        # x_loc free rows per channel
SHIFT = 7     # x_loc[j] = image row j + 64*h - SHIFT
N_CORES = 8
OYS = 128     # output rows per core
N_GROUPS = 8  # 16 oys per group

JC = J * C
RHS = 512.0   # fp8 storage scale for the routing hidden


def _q8(x):
    return np.ascontiguousarray(np.asarray(x, np.float32)).astype(md.float8_e4m3fn)


def _q16(x):
    return np.ascontiguousarray(np.asarray(x, np.float32)).astype(np.float16)


def _f32(x):
    return np.ascontiguousarray(np.asarray(x, np.float32))


# ----------------------------------------------------------------------------
# host-side reference math (for the static rel-coord tables)
# ----------------------------------------------------------------------------

def _cubic_weights(t):
    x = t + 1.0
    w0 = ((A * x - 5.0 * A) * x + 8.0 * A) * x - 4.0 * A
    w1 = ((A + 2.0) * t - (A + 3.0)) * t * t + 1.0
    s = 1.0 - t
    w2 = ((A + 2.0) * s - (A + 3.0)) * s * s + 1.0
    w3 = 1.0 - w0 - w1 - w2
    return np.stack([w0, w1, w2, w3], axis=-1)


def _grid_sample_bicubic_np(feat, gx, gy):
    Bn, Cn, Hn, Wn = feat.shape
    ix = ((gx + 1.0) * Wn - 1.0) * 0.5
    iy = ((gy + 1.0) * Hn - 1.0) * 0.5
    ix0 = np.floor(ix)
    iy0 = np.floor(iy)
    wx = _cubic_weights(ix - ix0)
    wy = _cubic_weights(iy - iy0)
    ix0 = ix0.astype(np.int32)
    iy0 = iy0.astype(np.int32)
    ff = feat.reshape(Bn, Cn, Hn * Wn)
    out = np.zeros((Bn, Cn, gx.shape[1]), feat.dtype)
    for i in range(4):
        yi = iy0 - 1 + i
        yok = (yi >= 0) & (yi < Hn)
        yc = np.clip(yi, 0, Hn - 1)
        for jj in range(4):
            xj = ix0 - 1 + jj
            ok = yok & (xj >= 0) & (xj < Wn)
            xc = np.clip(xj, 0, Wn - 1)
            v = np.take_along_axis(ff, (yc * Wn + xc)[:, None, :], axis=2)
            w = wy[..., i] * wx[..., jj] * ok
            out = out + v * w[:, None, :].astype(feat.dtype)
    return out


def _make_coord(Hn, Wn):
    y = -1.0 + (2.0 * np.arange(Hn, dtype=np.float32) + 1.0) / Hn
    x = -1.0 + (2.0 * np.arange(Wn, dtype=np.float32) + 1.0) / Wn
    yy, xx = np.meshgrid(y, x, indexing="ij")
    return np.stack([yy, xx], axis=-1).astype(np.float32)


def _rel_table():
    """rel features [Q, 4] = [rel_coord_y, rel_coord_x, rel_cell_y, rel_cell_x]."""
    Q = HO * WO
    coord = _make_coord(HO, WO).reshape(Q, 2)
    cell = np.ones((Q, 2), np.float32)
    cell[0] *= 2.0 / HO
    cell[1] *= 2.0 / WO
    cc = np.clip(coord, -1.0 + 1e-6, 1.0 - 1e-6)
    gy, gx = cc[None, :, 0], cc[None, :, 1]
    fc = np.broadcast_to(_make_coord(H, W).transpose(2, 0, 1)[None], (1, 2, H, W))
    q_coord = _grid_sample_bicubic_np(fc, gx, gy).transpose(0, 2, 1)[0]
    hw = np.array([H, W], np.float32)
    rel_coord = (coord - q_coord) * hw
    rel_cell = cell * hw
    return np.concatenate([rel_coord, rel_cell], axis=-1).astype(np.float32)


def _ky_l(oy_l):
    return int(np.floor(oy_l / 2.0 - 0.25))


def _s_prog(oy_l):
    return _ky_l(oy_l) + 4


# ----------------------------------------------------------------------------
# static tables (shared across cores)
# ----------------------------------------------------------------------------

def _build_shared(inputs):
    rel = _rel_table()

    w0 = _f32(inputs["mlp_w0"])
    b0 = _f32(inputs["mlp_b0"])
    w1 = _f32(inputs["mlp_w1"])
    b1 = _f32(inputs["mlp_b1"])
    w2 = _f32(inputs["mlp_w2"])
    b2 = _f32(inputs["mlp_b2"])
    rw0 = _f32(inputs["rout_w0"])
    rb0 = _f32(inputs["rout_b0"])
    rw1 = _f32(inputs["rout_w1"])
    rb1 = _f32(inputs["rout_b1"])
    ow0 = _f32(inputs["off_w0"])
    ob0 = _f32(inputs["off_b0"])
    ow1 = _f32(inputs["off_w1"])
    b_off = _f32(inputs["off_b1"])

    bandx0 = np.zeros((W, WO), np.float32)
    for ox in range(WO):
        ix = ox / 2.0 - 0.25
        kx = int(np.floor(ix))
        wts = _cubic_weights(np.float32(ix - kx))
        for tap in range(4):
            wc = kx - 1 + tap
            if 0 <= wc < W:
                bandx0[wc, ox] = wts[tap]
    # first sample fused vertical x horizontal: bandq8[w, par, rp, r', ox]
    # = wy0[par][2rp+r'] * bandx0[w, ox]; DR moving operand with x_loc8f
    # j-slices stationary.
    bandq8 = np.zeros((W, 2, 2, 2, WO), np.float32)
    for par, t in ((0, 0.75), (1, 0.25)):
        wy0 = _cubic_weights(np.float32(t))
        for rp in range(2):
            for r2 in range(2):
                bandq8[:, par, rp, r2, :] = wy0[2 * rp + r2] * bandx0

    # l1: K = 37 (32 qf + 4 rel + ones row)
    w0b = np.zeros((37, 256), np.float32)
    w0b[0:36] = w0
    w0b[36] = b0

    # l2: K = 2x128 fp16 (kh-sliced)
    w1p = np.zeros((128, 2, 256), np.float32)
    for kh in range(2):
        w1p[:, kh, :] = w1[kh * 128:(kh + 1) * 128, :]

    w2p = np.zeros((128, 2, 32), np.float32)
    for kh in range(2):
        w2p[:, kh, :] = w2[kh * 128:(kh + 1) * 128, :]

    # heads hidden: K = 33 (row 32 = ones -> bias)
    rw0b = np.zeros((33, 256), np.float32)
    rw0b[0:32] = rw0
    rw0b[32] = rb0
    ow0b = np.zeros((33, 256), np.float32)
    ow0b[0:32] = ow0
    ow0b[32] = ob0

    # routing out: DR over fp8 rhid stored at 512x -> rout_ps = 4096*routlin
    rw1dr = np.zeros((128, 2, 32), np.float32)
    for kt in range(2):
        rw1dr[:, kt, :] = 8.0 * rw1[kt * 128:(kt + 1) * 128, :]
    # offset out: fp16, x -> partition 0, y -> partition 32
    ow1p = np.zeros((128, 2, 64), np.float32)
    for kh in range(2):
        ow1p[:, kh, 0] = ow1[kh * 128:(kh + 1) * 128, 0]
        ow1p[:, kh, 32] = ow1[kh * 128:(kh + 1) * 128, 1]

    # static distance table for the offset horizontal band (b_off folded),
    # clipped to keep the fp16 eval in range (clipped taps evaluate to 0)
    ox = np.arange(256, dtype=np.float32)
    ixb = ox / 2.0 - 0.25 + 64.0 * b_off[0]
    dcol = np.clip(np.arange(W, dtype=np.float32)[:, None] - ixb[None, :], -8.0, 8.0)
    dstatp = np.tile(dcol, (1, 4))        # pair level [128, 1024]

    # vertical-weight selector; 1.25 compensates the Ky eval's 1/1.25
    sel = np.zeros((128, 32 * 128), np.float32)
    for half in range(2):
        for o in range(16):
            Sm = np.zeros((128, 128), np.float32)
            for m in range(128):
                r2 = half * 4 + m // 32
                Sm[o * 8 + r2, m] = 1.25
            sel[:, (half * 16 + o) * 128:(half * 16 + o + 1) * 128] = Sm

    sumc = np.zeros((128, 32), np.float32)
    sumc[np.arange(128), np.arange(128) % 32] = 1.0

    oybcast = np.zeros((16, 128), np.float32)
    oybcast[np.arange(128) // 8, np.arange(128)] = 1.0

    static = {
        "bandq8": _q8(bandq8.reshape(128, 2048)),
        "w0b": _q16(w0b),
        "w1p": _q16(w1p.reshape(128, 512)),
        "w2p": _q16(w2p.reshape(128, 64)),
        "rw0b": _q16(rw0b),
        "ow0b": _q16(ow0b),
        "rw1dr": _q8(rw1dr.reshape(128, 64)),
        "ow1p": _q16(ow1p.reshape(128, 128)),
        "b1a": _f32(b1[0:128].reshape(128, 1)),
        "b1b": _f32(b1[128:256].reshape(128, 1)),
        "b2c": _f32(b2.reshape(32, 1)),
        "rb1p1": _f32((1.0 + rb1).reshape(32, 1)),
        "dstatp": _q16(dstatp),
        "sel": _q16(sel),
        "sumc16": _q16(sumc),
        "oybc64": _f32(64.0 * oybcast),
        "onesm": np.full((1, 128), -64.0, np.float32),
    }
    return {"rel": rel, "b_off": b_off, "static": static}


# ----------------------------------------------------------------------------
# per-core input tables
# ----------------------------------------------------------------------------

def _build_core_inputs(inputs, b, h, shared):
    x = np.asarray(inputs["x"])[b]
    x_loc = np.zeros((W, J, C), np.float32)
    rows = np.arange(J) + 64 * h - SHIFT
    valid = (rows >= 0) & (rows < H)
    x_loc[:, valid, :] = x[:, rows[valid], :].transpose(2, 1, 0)
    x_loc = x_loc.reshape(W, JC)

    b_off = shared["b_off"]
    # candcn[(o,r), g] = -(candpos - iy(oy) - 64*b_off_y); Act-bias form so
    # dy = 64*offy + candcn in one op (K is even in d; sentinel 16 -> K = 0)
    candcn = np.zeros((128, 8), np.float32)
    for g in range(8):
        for o in range(16):
            oy_g = h * 128 + 16 * g + o
            iy = oy_g / 2.0 - 0.25
            ky = int(np.floor(iy))
            for r in range(8):
                row = ky - 3 + r
                candcn[o * 8 + r, g] = -(
                    row - iy - 64.0 * b_off[1] if 0 <= row < H else 16.0)

    q0 = h * 128 * 256
    # kt rows 32..36 of the l1 input: 4 rel features + the bias-ones row
    relrows = np.zeros((5, OYS * 256), np.float32)
    relrows[0:4] = shared["rel"][q0:q0 + OYS * 256].T
    relrows[4] = 1.0

    d = {
        # 1.25 compensates the Kx eval's 1/1.25 scaling (see _keval)
        "x_loc16": _q16(1.25 * x_loc),
        "x_loc8f": _q8(x_loc),
        "candcn": _f32(candcn),
        "relrows16": _q16(relrows),
    }
    d.update(shared["static"])
    return d


_SHAPES = {
    "x_loc16": ((W, JC), FP16),
    "x_loc8f": ((W, JC), FP8),
    "candcn": ((128, 8), F32),
    "relrows16": ((5, OYS * 256), FP16),
    "bandq8": ((128, 2048), FP8),
    "w0b": ((37, 256), FP16),
    "w1p": ((128, 512), FP16),
    "w2p": ((128, 64), FP16),
    "rw0b": ((33, 256), FP16),
    "ow0b": ((33, 256), FP16),
    "rw1dr": ((128, 64), FP8),
    "ow1p": ((128, 128), FP16),
    "b1a": ((128, 1), F32),
    "b1b": ((128, 1), F32),
    "b2c": ((32, 1), F32),
    "rb1p1": ((32, 1), F32),
    "dstatp": ((128, 1024), FP16),
    "sel": ((128, 32 * 128), FP16),
    "sumc16": ((128, 32), FP16),
    "oybc64": ((16, 128), F32R),
    "onesm": ((1, 128), F32R),
}

# tiles reshaped to >2D on-chip
_TILE3D = {
    "x_loc8f": (W, J, C),
    "bandq8": (W, 2, 2, 2, 256),
    "w1p": (128, 2, 256),
    "w2p": (128, 2, 32),
    "rw1dr": (128, 2, 32),
    "ow1p": (128, 2, 64),
}


def _fix_excess_waits(nc):
    """This walrus build allows only ONE semaphore wait per instruction.

    For any instruction carrying more, move the extra waits onto fresh NOPs
    inserted immediately before it on the same engine (identical semantics:
    the engine blocks on the same waits at the same program point).
    """
    blocks = list(nc.main_func.blocks)
    for bb in blocks:
        insts = bb.instructions
        i = 0
        while i < len(insts):
            ins = insts[i]
            si = ins.sync_info
            if si is not None and len(si.on_wait) > 1:
                waits = list(si.on_wait)
                extra, keep = waits[:-1], waits[-1:]
                nops = []
                for w in extra:
                    nop = nc.engines[ins.engine].nop(nofuse=True,
                                                     hint="wsplit").ins
                    for obb in blocks:
                        try:
                            obb.instructions.remove(nop)
                            break
                        except ValueError:
                            continue
                    nop.sync_info = mybir.SyncInfo(on_wait=[w], on_update=[])
                    nops.append(nop)
                ins.sync_info = mybir.SyncInfo(on_wait=keep,
                                               on_update=list(si.on_update))
                insts[i:i] = nops
                i += len(nops)
            i += 1


def _keval(nc, wp, et, nq, tag, out_bufs=2):
    """Exact bicubic kernel K(|et|)/1.25 -> fp16 tile [128, nq].

    K/1.25 = min(sa + (0.8 - 1.8 s), (-0.6 a + 0.6)(min(a,2)-2)^2), s = a^2,
    sa = s*a. Consumers' static weights carry the 1.25 back. Valid for the
    pre-clipped |et| <= ~16 range (fp16-safe).
    """
    a = wp.tile([128, nq], FP16, tag=f"{tag}a", bufs=3)
    s = wp.tile([128, nq], FP16, tag=f"{tag}s", bufs=3)
    c1 = wp.tile([128, nq], FP16, tag=f"{tag}c1", bufs=3)
    sa = wp.tile([128, nq], FP16, tag=f"{tag}sa", bufs=3)
    p1 = wp.tile([128, nq], FP16, tag=f"{tag}p1", bufs=3)
    t2 = wp.tile([128, nq], FP16, tag=f"{tag}t2", bufs=3)
    q2 = wp.tile([128, nq], FP16, tag=f"{tag}q2", bufs=3)
    r1 = wp.tile([128, nq], FP16, tag=f"{tag}r1", bufs=3)
    m2 = wp.tile([128, nq], FP16, tag=f"{tag}m2", bufs=3)
    kt = wp.tile([128, nq], FP16, tag=f"{tag}kt", bufs=out_bufs)
    nc.scalar.activation(a[:, :], et[:, :], AF.Abs)
    nc.vector.tensor_tensor(s[:, :], et[:, :], et[:, :], ALU.mult)
    nc.gpsimd.tensor_scalar(c1[:, :], s[:, :], -1.8, 0.8, ALU.mult, ALU.add)
    nc.vector.tensor_tensor(sa[:, :], s[:, :], a[:, :], ALU.mult)
    nc.vector.tensor_tensor(p1[:, :], sa[:, :], c1[:, :], ALU.add)
    nc.gpsimd.tensor_scalar(t2[:, :], a[:, :], -2.0, 0.0, ALU.add, ALU.min)
    nc.vector.tensor_tensor(q2[:, :], t2[:, :], t2[:, :], ALU.mult)
    nc.gpsimd.tensor_scalar(r1[:, :], a[:, :], -0.6, 0.6, ALU.mult, ALU.add)
    nc.vector.tensor_tensor(m2[:, :], r1[:, :], q2[:, :], ALU.mult)
    nc.vector.tensor_tensor(kt[:, :], p1[:, :], m2[:, :], ALU.min)
    return kt


def _build_program():
    nc = bass.Bass()
    P = {n: nc.declare_dram_parameter(n, list(s), d, isOutput=False)
         for n, (s, d) in _SHAPES.items()}
    outp = nc.declare_dram_parameter("outp", [C, OYS * 256], F32, isOutput=True)

    with tile.TileContext(nc) as tc:
        with (
            tc.tile_pool(name="consts", bufs=1) as cp,
            tc.tile_pool(name="work", bufs=2) as wp,
            tc.tile_pool(name="psM", bufs=2, space="PSUM") as psM,
            tc.tile_pool(name="psQ", bufs=2, space="PSUM") as psQ,
            tc.tile_pool(name="psG", bufs=2, space="PSUM") as psG,
        ):
            ct = {}
            for n, (s, d) in _SHAPES.items():
                if n == "relrows16":
                    continue  # streamed from DRAM per chunk
                shape = list(_TILE3D.get(n, s))
                t = cp.tile(shape, d, tag=n, name=n + "_sb")
                full = tuple(slice(None) for _ in shape)
                nc.gpsimd.dma_start(out=t[full], in_=P[n][:, :])
                ct[n] = t

            state = {}
            # persistent preds buffers (manual rotation): the bias-ones row
            # (partition 32) is written once and must survive reuse, which a
            # rotating pool-tile generation would flag as a stale read
            preds_bufs = []
            for i in range(3):
                t = cp.tile([33, 512], FP16, tag=f"preds{i}",
                            name=f"preds{i}")
                nc.gpsimd.memset(t[32:33, :], 1.0)
                preds_bufs.append(t)

            def chunk_mlp(g, cc):
                ch = g * 8 + cc
                sub = cc & 1
                oyA, oyB = 2 * ch, 2 * ch + 1
                y16 = state["y16"]

                # ---------- first sample: fused vertical x horizontal DR
                qf_ps = psQ.tile([32, 512], F32, tag="psQ", name="qf_ps")
                for t_i, oy in ((0, oyA), (1, oyB)):
                    sp = _s_prog(oy)
                    par = oy & 1
                    for rp in range(2):
                        j0 = sp + 2 + 2 * rp
                        nc.tensor.matmul(
                            qf_ps[:, t_i * 256:(t_i + 1) * 256],
                            ct["x_loc8f"][:, j0:j0 + 2, :],
                            ct["bandq8"][:, par, rp, :, :],
                            start=(rp == 0), stop=(rp == 1), perf_mode=DR)

                inp = wp.tile([37, 512], FP16, tag="inp", bufs=4,
                              name=f"inp_{ch}")
                nc.scalar.copy(inp[0:32, :], qf_ps[:, :])
                nc.sync.dma_start(
                    out=inp[32:37, :],
                    in_=P["relrows16"][:, ch * 512:(ch + 1) * 512])

                # ---------- MLP trunk (fp16)
                l1ps = psM.tile([128, 1024], F32, tag="psM", name="l1ps")
                for mh in range(2):
                    nc.tensor.matmul(l1ps[:, mh * 512:(mh + 1) * 512],
                                     ct["w0b"][:, mh * 128:(mh + 1) * 128],
                                     inp[:, :], start=True, stop=True)
                h1 = wp.tile([128, 2, 512], FP16, tag="h1", bufs=3, name="h1")
                nc.scalar.activation(h1[:, 0, :], l1ps[:, 0:512], AF.Relu)
                nc.vector.tensor_scalar(h1[:, 1, :], l1ps[:, 512:1024], 0.0,
                                        None, ALU.max)

                l2ps = psM.tile([128, 1024], F32, tag="psM", name="l2ps")
                for mh in range(2):
                    for kh in range(2):
                        nc.tensor.matmul(
                            l2ps[:, mh * 512:(mh + 1) * 512],
                            ct["w1p"][:, kh, mh * 128:(mh + 1) * 128],
                            h1[:, kh, :], start=(kh == 0), stop=(kh == 1))
                h2 = wp.tile([128, 2, 512], FP16, tag="h2", bufs=3, name="h2")
                nc.scalar.activation(h2[:, 0, :], l2ps[:, 0:512], AF.Relu,
                                     bias=ct["b1a"][:, 0:1])
                nc.vector.tensor_scalar(h2[:, 1, :], l2ps[:, 512:1024],
                                        ct["b1b"][:, 0:1], 0.0,
                                        ALU.add, ALU.max)

                pred_ps = psQ.tile([32, 512], F32, tag="psQ", name="pred_ps")
                for kh in range(2):
                    nc.tensor.matmul(pred_ps[:, :], ct["w2p"][:, kh, :],
                                     h2[:, kh, :],
                                     start=(kh == 0), stop=(kh == 1))
                preds = preds_bufs[ch % 3]
                nc.scalar.activation(preds[0:32, :], pred_ps[:, :],
                                     AF.Identity, bias=ct["b2c"][:, 0:1])

                # ---------- heads
                rhps = psM.tile([128, 1024], F32, tag="psM", name="rhps")
                for mh in range(2):
                    nc.tensor.matmul(rhps[:, mh * 512:(mh + 1) * 512],
                                     ct["rw0b"][:, mh * 128:(mh + 1) * 128],
                                     preds[:, :], start=True, stop=True)
                rhid = wp.tile([128, 2, 512], FP8, tag="rhid", bufs=3, name="rhid")
                nc.scalar.activation(rhid[:, 0, :], rhps[:, 0:512], AF.Relu,
                                     scale=RHS)
                nc.vector.tensor_scalar(rhid[:, 1, :], rhps[:, 512:1024],
                                        RHS, 0.0, ALU.mult, ALU.max)

                ohps = psM.tile([128, 1024], F32, tag="psM", name="ohps")
                for mh in range(2):
                    nc.tensor.matmul(ohps[:, mh * 512:(mh + 1) * 512],
                                     ct["ow0b"][:, mh * 128:(mh + 1) * 128],
                                     preds[:, :], start=True, stop=True)
                ohid = wp.tile([128, 2, 512], FP16, tag="ohid", bufs=3, name="ohid")
                nc.scalar.activation(ohid[:, 0, :], ohps[:, 0:512], AF.Relu)
                nc.vector.tensor_scalar(ohid[:, 1, :], ohps[:, 512:1024], 0.0,
                                        None, ALU.max)

                rout_ps = psQ.tile([32, 512], F32, tag="psQ", name="rout_ps")
                nc.tensor.matmul(rout_ps[:, :], ct["rw1dr"][:, :, :],
                                 rhid[:, :, :], start=True, stop=True,
                                 perf_mode=DR)
                rpo = wp.tile([32, 512], FP16, tag="rpo", bufs=20,
                              name=f"rpo_{ch}")
                nc.scalar.activation(rpo[:, :], rout_ps[:, :], AF.Identity,
                                     bias=ct["rb1p1"][:, 0:1],
                                     scale=1.0 / 4096.0)
                state["rpo"][ch] = rpo

                off_ps = psQ.tile([64, 512], F32, tag="psQ", name="off_ps")
                for kh in range(2):
                    nc.tensor.matmul(off_ps[:, :], ct["ow1p"][:, kh, :],
                                     ohid[:, kh, :],
                                     start=(kh == 0), stop=(kh == 1))
                # raw offsets out of PSUM in one op (rows 0 = x, 32 = y)
                oxy = wp.tile([33, 512], F32R, tag="oxy", bufs=4, name="oxy")
                nc.scalar.copy(oxy[:, :], off_ps[0:33, :])
                nc.sync.dma_start(out=y16[2 * cc:2 * cc + 2, :],
                                  in_=oxy[32:33, :])

                # ---------- offset horizontal band (eval at chunk-pair level)
                bx_ps = psQ.tile([128, 512], F32, tag="psQ", name="bx_ps")
                nc.tensor.matmul(bx_ps[:, :], ct["onesm"][:, :], oxy[0:1, :],
                                 start=True, stop=True)
                if sub == 0:
                    state["et"] = wp.tile([128, 1024], FP16, tag="bxet",
                                          bufs=3, name="bxet")
                nc.vector.tensor_tensor(
                    state["et"][:, sub * 512:(sub + 1) * 512], bx_ps[:, :],
                    ct["dstatp"][:, 0:512], ALU.add)
                if sub == 1:
                    state["ktp"][ch // 2] = _keval(nc, wp, state["et"], 1024,
                                                   "bx", out_bufs=10)

            def group_cwy(g):
                y16 = state["y16"]
                cwyin = psQ.tile([128, 256], F32, tag="psQ", name="cwyin")
                nc.tensor.matmul(cwyin[:, :], ct["oybc64"][:, :], y16[:, :],
                                 start=True, stop=True)
                dy = wp.tile([128, 256], FP16, tag="cwdy", bufs=3, name="cwdy")
                nc.scalar.activation(dy[:, :], cwyin[:, :], AF.Identity,
                                     bias=ct["candcn"][:, g:g + 1])
                state["cwyp"][g] = _keval(nc, wp, dy, 256, "cw", out_bufs=3)

            def chunk_gather(g, cc):
                ch = g * 8 + cc
                sub = cc & 1
                oyA, oyB = 2 * ch, 2 * ch + 1
                spA, spB = _s_prog(oyA), _s_prog(oyB)
                ktp = state["ktp"][ch // 2]
                cwyp = state["cwyp"][g]

                out0 = psQ.tile([32, 512], F32, tag="psQ", name="out0")
                for half in range(2):
                    hgps = psG.tile([128, 512], F32, tag="psG", name="hgps")
                    for t_i, sp in ((0, spA), (1, spB)):
                        nc.tensor.matmul(
                            hgps[:, t_i * 256:t_i * 256 + 256],
                            ct["x_loc16"][:, sp * 32 + half * 128:
                                          sp * 32 + half * 128 + 128],
                            ktp[:, sub * 512 + t_i * 256:
                                sub * 512 + t_i * 256 + 256],
                            start=True, stop=True)
                    wyps = psG.tile([128, 512], F32, tag="psG", name="wyps")
                    for t_i in range(2):
                        o = (2 * cc + t_i) % 16
                        si = (half * 16 + o) * 128
                        nc.tensor.matmul(
                            wyps[:, t_i * 256:t_i * 256 + 256],
                            ct["sel"][:, si:si + 128],
                            cwyp[:, :], start=True, stop=True)
                    wys = wp.tile([128, 512], FP16, tag="wys", bufs=6,
                                  name="wys")
                    if half == 0:
                        nc.scalar.copy(wys[:, :], wyps[:, :])
                    else:
                        nc.vector.tensor_scalar(wys[:, :], wyps[:, :], 1.0,
                                                None, ALU.mult)
                    hwt = wp.tile([128, 512], FP16, tag="hwt", bufs=6,
                                  name="hwt")
                    nc.vector.tensor_tensor(hwt[:, :], hgps[:, :], wys[:, :],
                                            ALU.mult)
                    nc.tensor.matmul(
                        out0[:, :], ct["sumc16"][:, :], hwt[:, :],
                        start=(half == 0), stop=(half == 1))
                if sub == 0:
                    state["outtp"] = wp.tile([32, 1024], F32, tag="outtp", bufs=3,
                                             name="outtp")
                nc.vector.tensor_tensor(
                    state["outtp"][:, sub * 512:(sub + 1) * 512], out0[:, :],
                    state["rpo"][ch][:, :], ALU.mult)
                if sub == 1:
                    nc.sync.dma_start(
                        out=outp[:, (ch - 1) * 512:(ch + 1) * 512],
                        in_=state["outtp"][:, :])

            # software pipeline: group g's MLP interleaves with group g-1's
            # gather at chunk granularity
            state["rpo"] = {}
            state["ktp"] = {}
            state["cwyp"] = {}
            for g in range(N_GROUPS):
                state["y16"] = wp.tile([16, 256], F32R, tag="y16", bufs=3,
                                       name=f"y16_{g}")
                for cc in range(8):
                    chunk_mlp(g, cc)
                    if g > 0:
                        chunk_gather(g - 1, cc)
                group_cwy(g)
            for cc in range(8):
                chunk_gather(N_GROUPS - 1, cc)

    _fix_excess_waits(nc)
    return nc


_PROGRAM = None
_LAST_EXEC_NS = None


def kernel(**inputs):
    global _PROGRAM
    if _PROGRAM is None:
        _PROGRAM = _build_program()
    nc = _PROGRAM
    shared = _build_shared(inputs)
    in_maps = []
    for core in range(N_CORES):
        b, h = divmod(core, 2)
        in_maps.append(_build_core_inputs(inputs, b, h, shared))
    trace = os.environ.get("KTRACE", "0") == "1"
    try:
        res = run_bass_kernel_spmd(nc, in_maps, list(range(N_CORES)),
                                   trace=trace)
    except Exception:
        if not trace:
            raise
        res = run_bass_kernel_spmd(nc, in_maps, list(range(N_CORES)))
    global _LAST_EXEC_NS
    _LAST_EXEC_NS = res.exec_time_ns
    out = np.zeros((B, C, HO, WO), np.float32)
    for core in range(N_CORES):
        b, h = divmod(core, 2)
        o = res.results[core]["outp"].reshape(C, OYS, 256)
        out[b, :, h * 128:h * 128 + 128, :] = o
    return out


# revision 43
# speedup vs baseline: 1.0341x; 1.0227x over previous
"""Trainium2 Bass kernel for nn_CRM_14886356648008 (LIIF-style SR module).

Sharding: 8 cores = 4 images x 2 output-row halves. Each core computes
out[b, :, h*128:(h+1)*128, :] from the full input image plus static tables.

v3 design (vs the f32r baseline at ~850us):
  - First (regular-grid) bicubic sample as fp8 DoubleRow matmuls with the
    vertical taps fused into static per-parity weights (the MLP's *feature*
    input tolerates fp8; the offset path does not care about it).
  - MLP trunk + offset head in fp16 (the predicted sample offset needs
    ~1% accuracy because d(out)/d(offset) reaches ~4 per pixel); routing
    head output in fp8 with a DoubleRow head matmul (routing tolerates it).
  - Per-query biases ride inside matmuls as spare contraction rows carrying
    ones, so PSUM->SBUF conversions are single fused activation ops.
  - Bicubic weights evaluated exactly via
        K/1.25 = min(sa + (0.8 - 1.8 s), (-0.6 a + 0.6)(min(a,2)-2)^2),
    s = a^2, sa = s*a, in fp16 on [128, 1024] chunk-pair tiles; the 1.25
    is folded into the gather's static operands. Static distance tables are
    pre-clipped so fp16 never overflows (clipped taps evaluate to exactly 0).
  - Vertical-weight broadcast via per-(oy,half) one-hot selector matmuls.
  - Group-level software pipelining: group g's MLP work is interleaved with
    group g-1's gather at chunk granularity so the in-order engine streams
    never drain at phase boundaries.
"""
import os
import numpy as np
import ml_dtypes as md

import concourse.bass as bass
import concourse.tile as tile
from concourse import mybir
from concourse.bass_utils import run_bass_kernel_spmd

F32 = mybir.dt.float32
F32R = mybir.dt.float32r
FP16 = mybir.dt.float16
FP8 = mybir.dt.float8e4
AF = mybir.ActivationFunctionType
ALU = mybir.AluOpType
DR = mybir.MatmulPerfMode.DoubleRow

A = -0.75
B, C, H, W = 4, 32, 128, 128
SCALE = 2
HO, WO = H * SCALE, W * SCALE
J = 76        # x_loc free rows per channel
SHIFT = 7     # x_loc[j] = image row j + 64*h - SHIFT
N_CORES = 8
OYS = 128     # output rows per core
N_GROUPS = 8  # 16 oys per group

JC = J * C
RHS = 512.0   # fp8 storage scale for the routing hidden


def _q8(x):
    return np.ascontiguousarray(np.asarray(x, np.float32)).astype(md.float8_e4m3fn)


def _q16(x):
    return np.ascontiguousarray(np.asarray(x, np.float32)).astype(np.float16)


def _f32(x):
    return np.ascontiguousarray(np.asarray(x, np.float32))


# ----------------------------------------------------------------------------
# host-side reference math (for the static rel-coord tables)
# ----------------------------------------------------------------------------

def _cubic_weights(t):
    x = t + 1.0
    w0 = ((A * x - 5.0 * A) * x + 8.0 * A) * x - 4.0 * A
    w1 = ((A + 2.0) * t - (A + 3.0)) * t * t + 1.0
    s = 1.0 - t
    w2 = ((A + 2.0) * s - (A + 3.0)) * s * s + 1.0
    w3 = 1.0 - w0 - w1 - w2
    return np.stack([w0, w1, w2, w3], axis=-1)


def _grid_sample_bicubic_np(feat, gx, gy):
    Bn, Cn, Hn, Wn = feat.shape
    ix = ((gx + 1.0) * Wn - 1.0) * 0.5
    iy = ((gy + 1.0) * Hn - 1.0) * 0.5
    ix0 = np.floor(ix)
    iy0 = np.floor(iy)
    wx = _cubic_weights(ix - ix0)
    wy = _cubic_weights(iy - iy0)
    ix0 = ix0.astype(np.int32)
    iy0 = iy0.astype(np.int32)
    ff = feat.reshape(Bn, Cn, Hn * Wn)
    out = np.zeros((Bn, Cn, gx.shape[1]), feat.dtype)
    for i in range(4):
        yi = iy0 - 1 + i
        yok = (yi >= 0) & (yi < Hn)
        yc = np.clip(yi, 0, Hn - 1)
        for jj in range(4):
            xj = ix0 - 1 + jj
            ok = yok & (xj >= 0) & (xj < Wn)
            xc = np.clip(xj, 0, Wn - 1)
            v = np.take_along_axis(ff, (yc * Wn + xc)[:, None, :], axis=2)
            w = wy[..., i] * wx[..., jj] * ok
            out = out + v * w[:, None, :].astype(feat.dtype)
    return out


def _make_coord(Hn, Wn):
    y = -1.0 + (2.0 * np.arange(Hn, dtype=np.float32) + 1.0) / Hn
    x = -1.0 + (2.0 * np.arange(Wn, dtype=np.float32) + 1.0) / Wn
    yy, xx = np.meshgrid(y, x, indexing="ij")
    return np.stack([yy, xx], axis=-1).astype(np.float32)


def _rel_table():
    """rel features [Q, 4] = [rel_coord_y, rel_coord_x, rel_cell_y, rel_cell_x]."""
    Q = HO * WO
    coord = _make_coord(HO, WO).reshape(Q, 2)
    cell = np.ones((Q, 2), np.float32)
    cell[0] *= 2.0 / HO
    cell[1] *= 2.0 / WO
    cc = np.clip(coord, -1.0 + 1e-6, 1.0 - 1e-6)
    gy, gx = cc[None, :, 0], cc[None, :, 1]
    fc = np.broadcast_to(_make_coord(H, W).transpose(2, 0, 1)[None], (1, 2, H, W))
    q_coord = _grid_sample_bicubic_np(fc, gx, gy).transpose(0, 2, 1)[0]
    hw = np.array([H, W], np.float32)
    rel_coord = (coord - q_coord) * hw
    rel_cell = cell * hw
    return np.concatenate([rel_coord, rel_cell], axis=-1).astype(np.float32)


def _ky_l(oy_l):
    return int(np.floor(oy_l / 2.0 - 0.25))


def _s_prog(oy_l):
    return _ky_l(oy_l) + 4


# ----------------------------------------------------------------------------
# static tables (shared across cores)
# ----------------------------------------------------------------------------

def _build_shared(inputs):
    rel = _rel_table()

    w0 = _f32(inputs["mlp_w0"])
    b0 = _f32(inputs["mlp_b0"])
    w1 = _f32(inputs["mlp_w1"])
    b1 = _f32(inputs["mlp_b1"])
    w2 = _f32(inputs["mlp_w2"])
    b2 = _f32(inputs["mlp_b2"])
    rw0 = _f32(inputs["rout_w0"])
    rb0 = _f32(inputs["rout_b0"])
    rw1 = _f32(inputs["rout_w1"])
    rb1 = _f32(inputs["rout_b1"])
    ow0 = _f32(inputs["off_w0"])
    ob0 = _f32(inputs["off_b0"])
    ow1 = _f32(inputs["off_w1"])
    b_off = _f32(inputs["off_b1"])

    bandx0 = np.zeros((W, WO), np.float32)
    for ox in range(WO):
        ix = ox / 2.0 - 0.25
        kx = int(np.floor(ix))
        wts = _cubic_weights(np.float32(ix - kx))
        for tap in range(4):
            wc = kx - 1 + tap
            if 0 <= wc < W:
                bandx0[wc, ox] = wts[tap]
    # first sample fused vertical x horizontal: bandq8[w, par, rp, r', ox]
    # = wy0[par][2rp+r'] * bandx0[w, ox]; DR moving operand with x_loc8f
    # j-slices stationary.
    bandq8 = np.zeros((W, 2, 2, 2, WO), np.float32)
    for par, t in ((0, 0.75), (1, 0.25)):
        wy0 = _cubic_weights(np.float32(t))
        for rp in range(2):
            for r2 in range(2):
                bandq8[:, par, rp, r2, :] = wy0[2 * rp + r2] * bandx0

    # l1: K = 37 (32 qf + 4 rel + ones row)
    w0b = np.zeros((37, 256), np.float32)
    w0b[0:36] = w0
    w0b[36] = b0

    # l2: K = 2x128 fp16 (kh-sliced)
    w1p = np.zeros((128, 2, 256), np.float32)
    for kh in range(2):
        w1p[:, kh, :] = w1[kh * 128:(kh + 1) * 128, :]

    # heads with the pred layer folded in: Wc = w2 @ w_head [256, 256],
    # bias' = b2 @ w_head + b_head, consumed straight from h2 (K = 2x128 fp16)
    Wcr = w2 @ rw0
    Wco = w2 @ ow0
    brh = b2 @ rw0 + rb0
    boh = b2 @ ow0 + ob0
    wcr = np.zeros((128, 2, 256), np.float32)
    wco = np.zeros((128, 2, 256), np.float32)
    for kh in range(2):
        wcr[:, kh, :] = Wcr[kh * 128:(kh + 1) * 128, :]
        wco[:, kh, :] = Wco[kh * 128:(kh + 1) * 128, :]

    # routing out: DR over fp8 rhid stored at 512x -> rout_ps = 4096*routlin
    rw1dr = np.zeros((128, 2, 32), np.float32)
    for kt in range(2):
        rw1dr[:, kt, :] = 8.0 * rw1[kt * 128:(kt + 1) * 128, :]
    # offset out: fp16, x -> partition 0, y -> partition 32
    ow1p = np.zeros((128, 2, 64), np.float32)
    for kh in range(2):
        ow1p[:, kh, 0] = ow1[kh * 128:(kh + 1) * 128, 0]
        ow1p[:, kh, 32] = ow1[kh * 128:(kh + 1) * 128, 1]

    # static distance table for the offset horizontal band (b_off folded),
    # clipped to keep the fp16 eval in range (clipped taps evaluate to 0)
    ox = np.arange(256, dtype=np.float32)
    ixb = ox / 2.0 - 0.25 + 64.0 * b_off[0]
    dcol = np.clip(np.arange(W, dtype=np.float32)[:, None] - ixb[None, :], -8.0, 8.0)
    dstatp = np.tile(dcol, (1, 4))        # pair level [128, 1024]

    # vertical-weight selector; 1.25 compensates the Ky eval's 1/1.25
    sel = np.zeros((128, 32 * 128), np.float32)
    for half in range(2):
        for o in range(16):
            Sm = np.zeros((128, 128), np.float32)
            for m in range(128):
                r2 = half * 4 + m // 32
                Sm[o * 8 + r2, m] = 1.25
            sel[:, (half * 16 + o) * 128:(half * 16 + o + 1) * 128] = Sm

    sumc = np.zeros((128, 32), np.float32)
    sumc[np.arange(128), np.arange(128) % 32] = 1.0

    oybcast = np.zeros((16, 128), np.float32)
    oybcast[np.arange(128) // 8, np.arange(128)] = 1.0

    static = {
        "bandq8": _q8(bandq8.reshape(128, 2048)),
        "w0b": _q16(w0b),
        "w1p": _q16(w1p.reshape(128, 512)),
        "wcr": _q16(wcr.reshape(128, 512)),
        "wco": _q16(wco.reshape(128, 512)),
        "brha": _f32(RHS * brh[0:128].reshape(128, 1)),
        "brhb": _f32(RHS * brh[128:256].reshape(128, 1)),
        "boha": _f32(boh[0:128].reshape(128, 1)),
        "bohb": _f32(boh[128:256].reshape(128, 1)),
        "rw1dr": _q8(rw1dr.reshape(128, 64)),
        "ow1p": _q16(ow1p.reshape(128, 128)),
        "b1a": _f32(b1[0:128].reshape(128, 1)),
        "b1b": _f32(b1[128:256].reshape(128, 1)),
        "rb1p1": _f32((1.0 + rb1).reshape(32, 1)),
        "dstatp": _q16(dstatp),
        "sel": _q16(sel),
        "sumc16": _q16(sumc),
        "oybc64": _f32(64.0 * oybcast),
        "onesm": np.full((1, 128), -64.0, np.float32),
    }
    return {"rel": rel, "b_off": b_off, "static": static}


# ----------------------------------------------------------------------------
# per-core input tables
# ----------------------------------------------------------------------------

def _build_core_inputs(inputs, b, h, shared):
    x = np.asarray(inputs["x"])[b]
    x_loc = np.zeros((W, J, C), np.float32)
    rows = np.arange(J) + 64 * h - SHIFT
    valid = (rows >= 0) & (rows < H)
    x_loc[:, valid, :] = x[:, rows[valid], :].transpose(2, 1, 0)
    x_loc = x_loc.reshape(W, JC)

    b_off = shared["b_off"]
    # candcn[(o,r), g] = -(candpos - iy(oy) - 64*b_off_y); Act-bias form so
    # dy = 64*offy + candcn in one op (K is even in d; sentinel 16 -> K = 0)
    candcn = np.zeros((128, 8), np.float32)
    for g in range(8):
        for o in range(16):
            oy_g = h * 128 + 16 * g + o
            iy = oy_g / 2.0 - 0.25
            ky = int(np.floor(iy))
            for r in range(8):
                row = ky - 3 + r
                candcn[o * 8 + r, g] = -(
                    row - iy - 64.0 * b_off[1] if 0 <= row < H else 16.0)

    q0 = h * 128 * 256
    # kt rows 32..36 of the l1 input: 4 rel features + the bias-ones row
    relrows = np.zeros((5, OYS * 256), np.float32)
    relrows[0:4] = shared["rel"][q0:q0 + OYS * 256].T
    relrows[4] = 1.0

    d = {
        # 1.25 compensates the Kx eval's 1/1.25 scaling (see _keval)
        "x_loc16": _q16(1.25 * x_loc),
        "x_loc8f": _q8(x_loc),
        "candcn": _f32(candcn),
        "relrows16": _q16(relrows),
    }
    d.update(shared["static"])
    return d


_SHAPES = {
    "x_loc16": ((W, JC), FP16),
    "x_loc8f": ((W, JC), FP8),
    "candcn": ((128, 8), F32),
    "relrows16": ((5, OYS * 256), FP16),
    "bandq8": ((128, 2048), FP8),
    "w0b": ((37, 256), FP16),
    "w1p": ((128, 512), FP16),
    "wcr": ((128, 512), FP16),
    "wco": ((128, 512), FP16),
    "brha": ((128, 1), F32),
    "brhb": ((128, 1), F32),
    "boha": ((128, 1), F32),
    "bohb": ((128, 1), F32),
    "rw1dr": ((128, 64), FP8),
    "ow1p": ((128, 128), FP16),
    "b1a": ((128, 1), F32),
    "b1b": ((128, 1), F32),
    "rb1p1": ((32, 1), F32),
    "dstatp": ((128, 1024), FP16),
    "sel": ((128, 32 * 128), FP16),
    "sumc16": ((128, 32), FP16),
    "oybc64": ((16, 128), F32R),
    "onesm": ((1, 128), F32R),
}

# tiles reshaped to >2D on-chip
_TILE3D = {
    "x_loc8f": (W, J, C),
    "bandq8": (W, 2, 2, 2, 256),
    "w1p": (128, 2, 256),
    "wcr": (128, 2, 256),
    "wco": (128, 2, 256),
    "rw1dr": (128, 2, 32),
    "ow1p": (128, 2, 64),
}


def _fix_excess_waits(nc):
    """This walrus build allows only ONE semaphore wait per instruction.

    For any instruction carrying more, move the extra waits onto fresh NOPs
    inserted immediately before it on the same engine (identical semantics:
    the engine blocks on the same waits at the same program point).
    """
    blocks = list(nc.main_func.blocks)
    for bb in blocks:
        insts = bb.instructions
        i = 0
        while i < len(insts):
            ins = insts[i]
            si = ins.sync_info
            if si is not None and len(si.on_wait) > 1:
                waits = list(si.on_wait)
                extra, keep = waits[:-1], waits[-1:]
                nops = []
                for w in extra:
                    nop = nc.engines[ins.engine].nop(nofuse=True,
                                                     hint="wsplit").ins
                    for obb in blocks:
                        try:
                            obb.instructions.remove(nop)
                            break
                        except ValueError:
                            continue
                    nop.sync_info = mybir.SyncInfo(on_wait=[w], on_update=[])
                    nops.append(nop)
                ins.sync_info = mybir.SyncInfo(on_wait=keep,
                                               on_update=list(si.on_update))
                insts[i:i] = nops
                i += len(nops)
            i += 1


def _keval(nc, wp, et, nq, tag, out_bufs=2):
    """Exact bicubic kernel K(|et|)/1.25 -> fp16 tile [128, nq].

    K/1.25 = min(sa + (0.8 - 1.8 s), (-0.6 a + 0.6)(min(a,2)-2)^2), s = a^2,
    sa = s*a. Consumers' static weights carry the 1.25 back. Valid for the
    pre-clipped |et| <= ~16 range (fp16-safe).
    """
    a = wp.tile([128, nq], FP16, tag=f"{tag}a", bufs=3)
    s = wp.tile([128, nq], FP16, tag=f"{tag}s", bufs=3)
    c1 = wp.tile([128, nq], FP16, tag=f"{tag}c1", bufs=3)
    sa = wp.tile([128, nq], FP16, tag=f"{tag}sa", bufs=3)
    p1 = wp.tile([128, nq], FP16, tag=f"{tag}p1", bufs=3)
    t2 = wp.tile([128, nq], FP16, tag=f"{tag}t2", bufs=3)
    q2 = wp.tile([128, nq], FP16, tag=f"{tag}q2", bufs=3)
    r1 = wp.tile([128, nq], FP16, tag=f"{tag}r1", bufs=3)
    m2 = wp.tile([128, nq], FP16, tag=f"{tag}m2", bufs=3)
    kt = wp.tile([128, nq], FP16, tag=f"{tag}kt", bufs=out_bufs)
    nc.scalar.activation(a[:, :], et[:, :], AF.Abs)
    nc.vector.tensor_tensor(s[:, :], et[:, :], et[:, :], ALU.mult)
    nc.gpsimd.tensor_scalar(c1[:, :], s[:, :], -1.8, 0.8, ALU.mult, ALU.add)
    nc.vector.tensor_tensor(sa[:, :], s[:, :], a[:, :], ALU.mult)
    nc.vector.tensor_tensor(p1[:, :], sa[:, :], c1[:, :], ALU.add)
    nc.gpsimd.tensor_scalar(t2[:, :], a[:, :], -2.0, 0.0, ALU.add, ALU.min)
    nc.vector.tensor_tensor(q2[:, :], t2[:, :], t2[:, :], ALU.mult)
    nc.gpsimd.tensor_scalar(r1[:, :], a[:, :], -0.6, 0.6, ALU.mult, ALU.add)
    nc.vector.tensor_tensor(m2[:, :], r1[:, :], q2[:, :], ALU.mult)
    nc.vector.tensor_tensor(kt[:, :], p1[:, :], m2[:, :], ALU.min)
    return kt


def _build_program():
    nc = bass.Bass()
    P = {n: nc.declare_dram_parameter(n, list(s), d, isOutput=False)
         for n, (s, d) in _SHAPES.items()}
    outp = nc.declare_dram_parameter("outp", [C, OYS * 256], F32, isOutput=True)

    with tile.TileContext(nc) as tc:
        with (
            tc.tile_pool(name="consts", bufs=1) as cp,
            tc.tile_pool(name="work", bufs=2) as wp,
            tc.tile_pool(name="psM", bufs=2, space="PSUM") as psM,
            tc.tile_pool(name="psQ", bufs=2, space="PSUM") as psQ,
            tc.tile_pool(name="psG", bufs=2, space="PSUM") as psG,
        ):
            ct = {}
            for n, (s, d) in _SHAPES.items():
                if n == "relrows16":
                    continue  # streamed from DRAM per chunk
                shape = list(_TILE3D.get(n, s))
                t = cp.tile(shape, d, tag=n, name=n + "_sb")
                full = tuple(slice(None) for _ in shape)
                nc.gpsimd.dma_start(out=t[full], in_=P[n][:, :])
                ct[n] = t

            state = {}

            def chunk_mlp(g, cc):
                ch = g * 8 + cc
                sub = cc & 1
                oyA, oyB = 2 * ch, 2 * ch + 1
                y16 = state["y16"]

                # ---------- first sample: fused vertical x horizontal DR
                qf_ps = psQ.tile([32, 512], F32, tag="psQ", name="qf_ps")
                for t_i, oy in ((0, oyA), (1, oyB)):
                    sp = _s_prog(oy)
                    par = oy & 1
                    for rp in range(2):
                        j0 = sp + 2 + 2 * rp
                        nc.tensor.matmul(
                            qf_ps[:, t_i * 256:(t_i + 1) * 256],
                            ct["x_loc8f"][:, j0:j0 + 2, :],
                            ct["bandq8"][:, par, rp, :, :],
                            start=(rp == 0), stop=(rp == 1), perf_mode=DR)

                inp = wp.tile([37, 512], FP16, tag="inp", bufs=4,
                              name=f"inp_{ch}")
                nc.scalar.copy(inp[0:32, :], qf_ps[:, :])
                nc.sync.dma_start(
                    out=inp[32:37, :],
                    in_=P["relrows16"][:, ch * 512:(ch + 1) * 512])

                # ---------- MLP trunk (fp16)
                l1ps = psM.tile([128, 1024], F32, tag="psM", name="l1ps")
                for mh in range(2):
                    nc.tensor.matmul(l1ps[:, mh * 512:(mh + 1) * 512],
                                     ct["w0b"][:, mh * 128:(mh + 1) * 128],
                                     inp[:, :], start=True, stop=True)
                h1 = wp.tile([128, 2, 512], FP16, tag="h1", bufs=3, name="h1")
                nc.scalar.activation(h1[:, 0, :], l1ps[:, 0:512], AF.Relu)
                nc.vector.tensor_scalar(h1[:, 1, :], l1ps[:, 512:1024], 0.0,
                                        None, ALU.max)

                l2ps = psM.tile([128, 1024], F32, tag="psM", name="l2ps")
                for mh in range(2):
                    for kh in range(2):
                        nc.tensor.matmul(
                            l2ps[:, mh * 512:(mh + 1) * 512],
                            ct["w1p"][:, kh, mh * 128:(mh + 1) * 128],
                            h1[:, kh, :], start=(kh == 0), stop=(kh == 1))
                h2 = wp.tile([128, 2, 512], FP16, tag="h2", bufs=3, name="h2")
                nc.scalar.activation(h2[:, 0, :], l2ps[:, 0:512], AF.Relu,
                                     bias=ct["b1a"][:, 0:1])
                nc.vector.tensor_scalar(h2[:, 1, :], l2ps[:, 512:1024],
                                        ct["b1b"][:, 0:1], 0.0,
                                        ALU.add, ALU.max)

                # ---------- heads (pred layer folded into Wc = w2 @ w_head)
                rhps = psM.tile([128, 1024], F32, tag="psM", name="rhps")
                for mh in range(2):
                    for kh in range(2):
                        nc.tensor.matmul(
                            rhps[:, mh * 512:(mh + 1) * 512],
                            ct["wcr"][:, kh, mh * 128:(mh + 1) * 128],
                            h2[:, kh, :], start=(kh == 0), stop=(kh == 1))
                rhid = wp.tile([128, 2, 512], FP8, tag="rhid", bufs=3, name="rhid")
                nc.scalar.activation(rhid[:, 0, :], rhps[:, 0:512], AF.Relu,
                                     bias=ct["brha"][:, 0:1], scale=RHS)
                nc.scalar.activation(rhid[:, 1, :], rhps[:, 512:1024], AF.Relu,
                                     bias=ct["brhb"][:, 0:1], scale=RHS)

                ohps = psM.tile([128, 1024], F32, tag="psM", name="ohps")
                for mh in range(2):
                    for kh in range(2):
                        nc.tensor.matmul(
                            ohps[:, mh * 512:(mh + 1) * 512],
                            ct["wco"][:, kh, mh * 128:(mh + 1) * 128],
                            h2[:, kh, :], start=(kh == 0), stop=(kh == 1))
                ohid = wp.tile([128, 2, 512], FP16, tag="ohid", bufs=3, name="ohid")
                nc.scalar.activation(ohid[:, 0, :], ohps[:, 0:512], AF.Relu,
                                     bias=ct["boha"][:, 0:1])
                nc.vector.tensor_scalar(ohid[:, 1, :], ohps[:, 512:1024],
                                        ct["bohb"][:, 0:1], 0.0,
                                        ALU.add, ALU.max)

                rout_ps = psQ.tile([32, 512], F32, tag="psQ", name="rout_ps")
                nc.tensor.matmul(rout_ps[:, :], ct["rw1dr"][:, :, :],
                                 rhid[:, :, :], start=True, stop=True,
                                 perf_mode=DR)
                rpo = wp.tile([32, 512], FP16, tag="rpo", bufs=20,
                              name=f"rpo_{ch}")
                nc.scalar.activation(rpo[:, :], rout_ps[:, :], AF.Identity,
                                     bias=ct["rb1p1"][:, 0:1],
                                     scale=1.0 / 4096.0)
                state["rpo"][ch] = rpo

                off_ps = psQ.tile([64, 512], F32, tag="psQ", name="off_ps")
                for kh in range(2):
                    nc.tensor.matmul(off_ps[:, :], ct["ow1p"][:, kh, :],
                                     ohid[:, kh, :],
                                     start=(kh == 0), stop=(kh == 1))
                # raw offsets out of PSUM in one op (rows 0 = x, 32 = y)
                oxy = wp.tile([33, 512], F32R, tag="oxy", bufs=4, name="oxy")
                nc.scalar.copy(oxy[:, :], off_ps[0:33, :])
                nc.sync.dma_start(out=y16[2 * cc:2 * cc + 2, :],
                                  in_=oxy[32:33, :])

                # ---------- offset horizontal band (eval at chunk-pair level)
                bx_ps = psQ.tile([128, 512], F32, tag="psQ", name="bx_ps")
                nc.tensor.matmul(bx_ps[:, :], ct["onesm"][:, :], oxy[0:1, :],
                                 start=True, stop=True)
                if sub == 0:
                    state["et"] = wp.tile([128, 1024], FP16, tag="bxet",
                                          bufs=3, name="bxet")
                nc.vector.tensor_tensor(
                    state["et"][:, sub * 512:(sub + 1) * 512], bx_ps[:, :],
                    ct["dstatp"][:, 0:512], ALU.add)
                if sub == 1:
                    state["ktp"][ch // 2] = _keval(nc, wp, state["et"], 1024,
                                                   "bx", out_bufs=10)

            def group_cwy(g):
                y16 = state["y16"]
                cwyin = psQ.tile([128, 256], F32, tag="psQ", name="cwyin")
                nc.tensor.matmul(cwyin[:, :], ct["oybc64"][:, :], y16[:, :],
                                 start=True, stop=True)
                dy = wp.tile([128, 256], FP16, tag="cwdy", bufs=3, name="cwdy")
                nc.scalar.activation(dy[:, :], cwyin[:, :], AF.Identity,
                                     bias=ct["candcn"][:, g:g + 1])
                state["cwyp"][g] = _keval(nc, wp, dy, 256, "cw", out_bufs=3)

            def chunk_gather(g, cc):
                ch = g * 8 + cc
                sub = cc & 1
                oyA, oyB = 2 * ch, 2 * ch + 1
                spA, spB = _s_prog(oyA), _s_prog(oyB)
                ktp = state["ktp"][ch // 2]
                cwyp = state["cwyp"][g]

                out0 = psQ.tile([32, 512], F32, tag="psQ", name="out0")
                for half in range(2):
                    hgps = psG.tile([128, 512], F32, tag="psG", name="hgps")
                    for t_i, sp in ((0, spA), (1, spB)):
                        nc.tensor.matmul(
                            hgps[:, t_i * 256:t_i * 256 + 256],
                            ct["x_loc16"][:, sp * 32 + half * 128:
                                          sp * 32 + half * 128 + 128],
                            ktp[:, sub * 512 + t_i * 256:
                                sub * 512 + t_i * 256 + 256],
                            start=True, stop=True)
                    wyps = psG.tile([128, 512], F32, tag="psG", name="wyps")
                    for t_i in range(2):
                        o = (2 * cc + t_i) % 16
                        si = (half * 16 + o) * 128
                        nc.tensor.matmul(
                            wyps[:, t_i * 256:t_i * 256 + 256],
                            ct["sel"][:, si:si + 128],
                            cwyp[:, :], start=True, stop=True)
                    wys = wp.tile([128, 512], FP16, tag="wys", bufs=6,
                                  name="wys")
                    if half == 0:
                        nc.scalar.copy(wys[:, :], wyps[:, :])
                    else:
                        nc.vector.tensor_scalar(wys[:, :], wyps[:, :], 1.0,
                                                None, ALU.mult)
                    hwt = wp.tile([128, 512], FP16, tag="hwt", bufs=6,
                                  name="hwt")
                    nc.vector.tensor_tensor(hwt[:, :], hgps[:, :], wys[:, :],
                                            ALU.mult)
                    nc.tensor.matmul(
                        out0[:, :], ct["sumc16"][:, :], hwt[:, :],
                        start=(half == 0), stop=(half == 1))
                if sub == 0:
                    state["outtp"] = wp.tile([32, 1024], F32, tag="outtp", bufs=3,
                                             name="outtp")
                nc.vector.tensor_tensor(
                    state["outtp"][:, sub * 512:(sub + 1) * 512], out0[:, :],
                    state["rpo"][ch][:, :], ALU.mult)
                if sub == 1:
                    nc.sync.dma_start(
                        out=outp[:, (ch - 1) * 512:(ch + 1) * 512],
                        in_=state["outtp"][:, :])

            # software pipeline: group g's MLP interleaves with group g-1's
            # gather at chunk granularity
            state["rpo"] = {}
            state["ktp"] = {}
            state["cwyp"] = {}
            for g in range(N_GROUPS):
                state["y16"] = wp.tile([16, 256], F32R, tag="y16", bufs=3,
                                       name=f"y16_{g}")
                for cc in range(8):
                    chunk_mlp(g, cc)
                    if g > 0:
                        chunk_gather(g - 1, cc)
                group_cwy(g)
            for cc in range(8):
                chunk_gather(N_GROUPS - 1, cc)

    _fix_excess_waits(nc)
    return nc


_PROGRAM = None
_LAST_EXEC_NS = None


def kernel(**inputs):
    global _PROGRAM
    if _PROGRAM is None:
        _PROGRAM = _build_program()
    nc = _PROGRAM
    shared = _build_shared(inputs)
    in_maps = []
    for core in range(N_CORES):
        b, h = divmod(core, 2)
        in_maps.append(_build_core_inputs(inputs, b, h, shared))
    trace = os.environ.get("KTRACE", "0") == "1"
    try:
        res = run_bass_kernel_spmd(nc, in_maps, list(range(N_CORES)),
                                   trace=trace)
    except Exception:
        if not trace:
            raise
        res = run_bass_kernel_spmd(nc, in_maps, list(range(N_CORES)))
    global _LAST_EXEC_NS
    _LAST_EXEC_NS = res.exec_time_ns
    out = np.zeros((B, C, HO, WO), np.float32)
    for core in range(N_CORES):
        b, h = divmod(core, 2)
        o = res.results[core]["outp"].reshape(C, OYS, 256)
        out[b, :, h * 128:h * 128 + 128, :] = o
    return out


# revision 44
# speedup vs baseline: 1.0589x; 1.0239x over previous
"""Trainium2 Bass kernel for nn_CRM_14886356648008 (LIIF-style SR module).

Sharding: 8 cores = 4 images x 2 output-row halves. Each core computes
out[b, :, h*128:(h+1)*128, :] from the full input image plus static tables.

v3 design (vs the f32r baseline at ~850us):
  - First (regular-grid) bicubic sample as fp8 DoubleRow matmuls with the
    vertical taps fused into static per-parity weights (the MLP's *feature*
    input tolerates fp8; the offset path does not care about it).
  - MLP trunk + offset head in fp16 (the predicted sample offset needs
    ~1% accuracy because d(out)/d(offset) reaches ~4 per pixel); routing
    head output in fp8 with a DoubleRow head matmul (routing tolerates it).
  - Per-query biases ride inside matmuls as spare contraction rows carrying
    ones, so PSUM->SBUF conversions are single fused activation ops.
  - Bicubic weights evaluated exactly via
        K/1.25 = min(sa + (0.8 - 1.8 s), (-0.6 a + 0.6)(min(a,2)-2)^2),
    s = a^2, sa = s*a, in fp16 on [128, 1024] chunk-pair tiles; the 1.25
    is folded into the gather's static operands. Static distance tables are
    pre-clipped so fp16 never overflows (clipped taps evaluate to exactly 0).
  - Vertical-weight broadcast via per-(oy,half) one-hot selector matmuls.
  - Group-level software pipelining: group g's MLP work is interleaved with
    group g-1's gather at chunk granularity so the in-order engine streams
    never drain at phase boundaries.
"""
import os
import numpy as np
import ml_dtypes as md

import concourse.bass as bass
import concourse.tile as tile
from concourse import mybir
from concourse.bass_utils import run_bass_kernel_spmd

F32 = mybir.dt.float32
F32R = mybir.dt.float32r
FP16 = mybir.dt.float16
FP8 = mybir.dt.float8e4
AF = mybir.ActivationFunctionType
ALU = mybir.AluOpType
DR = mybir.MatmulPerfMode.DoubleRow

A = -0.75
B, C, H, W = 4, 32, 128, 128
SCALE = 2
HO, WO = H * SCALE, W * SCALE
J = 76        # x_loc free rows per channel
SHIFT = 7     # x_loc[j] = image row j + 64*h - SHIFT
N_CORES = 8
OYS = 128     # output rows per core
N_GROUPS = 8  # 16 oys per group

JC = J * C
RHS = 512.0   # fp8 storage scale for the routing hidden


def _q8(x):
    return np.ascontiguousarray(np.asarray(x, np.float32)).astype(md.float8_e4m3fn)


def _q16(x):
    return np.ascontiguousarray(np.asarray(x, np.float32)).astype(np.float16)


def _f32(x):
    return np.ascontiguousarray(np.asarray(x, np.float32))


# ----------------------------------------------------------------------------
# host-side reference math (for the static rel-coord tables)
# ----------------------------------------------------------------------------

def _cubic_weights(t):
    x = t + 1.0
    w0 = ((A * x - 5.0 * A) * x + 8.0 * A) * x - 4.0 * A
    w1 = ((A + 2.0) * t - (A + 3.0)) * t * t + 1.0
    s = 1.0 - t
    w2 = ((A + 2.0) * s - (A + 3.0)) * s * s + 1.0
    w3 = 1.0 - w0 - w1 - w2
    return np.stack([w0, w1, w2, w3], axis=-1)


def _grid_sample_bicubic_np(feat, gx, gy):
    Bn, Cn, Hn, Wn = feat.shape
    ix = ((gx + 1.0) * Wn - 1.0) * 0.5
    iy = ((gy + 1.0) * Hn - 1.0) * 0.5
    ix0 = np.floor(ix)
    iy0 = np.floor(iy)
    wx = _cubic_weights(ix - ix0)
    wy = _cubic_weights(iy - iy0)
    ix0 = ix0.astype(np.int32)
    iy0 = iy0.astype(np.int32)
    ff = feat.reshape(Bn, Cn, Hn * Wn)
    out = np.zeros((Bn, Cn, gx.shape[1]), feat.dtype)
    for i in range(4):
        yi = iy0 - 1 + i
        yok = (yi >= 0) & (yi < Hn)
        yc = np.clip(yi, 0, Hn - 1)
        for jj in range(4):
            xj = ix0 - 1 + jj
            ok = yok & (xj >= 0) & (xj < Wn)
            xc = np.clip(xj, 0, Wn - 1)
            v = np.take_along_axis(ff, (yc * Wn + xc)[:, None, :], axis=2)
            w = wy[..., i] * wx[..., jj] * ok
            out = out + v * w[:, None, :].astype(feat.dtype)
    return out


def _make_coord(Hn, Wn):
    y = -1.0 + (2.0 * np.arange(Hn, dtype=np.float32) + 1.0) / Hn
    x = -1.0 + (2.0 * np.arange(Wn, dtype=np.float32) + 1.0) / Wn
    yy, xx = np.meshgrid(y, x, indexing="ij")
    return np.stack([yy, xx], axis=-1).astype(np.float32)


def _rel_table():
    """rel features [Q, 4] = [rel_coord_y, rel_coord_x, rel_cell_y, rel_cell_x]."""
    Q = HO * WO
    coord = _make_coord(HO, WO).reshape(Q, 2)
    cell = np.ones((Q, 2), np.float32)
    cell[0] *= 2.0 / HO
    cell[1] *= 2.0 / WO
    cc = np.clip(coord, -1.0 + 1e-6, 1.0 - 1e-6)
    gy, gx = cc[None, :, 0], cc[None, :, 1]
    fc = np.broadcast_to(_make_coord(H, W).transpose(2, 0, 1)[None], (1, 2, H, W))
    q_coord = _grid_sample_bicubic_np(fc, gx, gy).transpose(0, 2, 1)[0]
    hw = np.array([H, W], np.float32)
    rel_coord = (coord - q_coord) * hw
    rel_cell = cell * hw
    return np.concatenate([rel_coord, rel_cell], axis=-1).astype(np.float32)


def _ky_l(oy_l):
    return int(np.floor(oy_l / 2.0 - 0.25))


def _s_prog(oy_l):
    return _ky_l(oy_l) + 4


# ----------------------------------------------------------------------------
# static tables (shared across cores)
# ----------------------------------------------------------------------------

def _build_shared(inputs):
    rel = _rel_table()

    w0 = _f32(inputs["mlp_w0"])
    b0 = _f32(inputs["mlp_b0"])
    w1 = _f32(inputs["mlp_w1"])
    b1 = _f32(inputs["mlp_b1"])
    w2 = _f32(inputs["mlp_w2"])
    b2 = _f32(inputs["mlp_b2"])
    rw0 = _f32(inputs["rout_w0"])
    rb0 = _f32(inputs["rout_b0"])
    rw1 = _f32(inputs["rout_w1"])
    rb1 = _f32(inputs["rout_b1"])
    ow0 = _f32(inputs["off_w0"])
    ob0 = _f32(inputs["off_b0"])
    ow1 = _f32(inputs["off_w1"])
    b_off = _f32(inputs["off_b1"])

    bandx0 = np.zeros((W, WO), np.float32)
    for ox in range(WO):
        ix = ox / 2.0 - 0.25
        kx = int(np.floor(ix))
        wts = _cubic_weights(np.float32(ix - kx))
        for tap in range(4):
            wc = kx - 1 + tap
            if 0 <= wc < W:
                bandx0[wc, ox] = wts[tap]
    # first sample fused vertical x horizontal: bandq8[w, par, rp, r', ox]
    # = wy0[par][2rp+r'] * bandx0[w, ox]; DR moving operand with x_loc8f
    # j-slices stationary.
    bandq8 = np.zeros((W, 2, 2, 2, WO), np.float32)
    for par, t in ((0, 0.75), (1, 0.25)):
        wy0 = _cubic_weights(np.float32(t))
        for rp in range(2):
            for r2 in range(2):
                bandq8[:, par, rp, r2, :] = wy0[2 * rp + r2] * bandx0

    # l1: K = 37 (32 qf + 4 rel + ones row)
    w0b = np.zeros((37, 256), np.float32)
    w0b[0:36] = w0
    w0b[36] = b0

    # l2: K = 2x128 fp16 (kh-sliced)
    w1p = np.zeros((128, 2, 256), np.float32)
    for kh in range(2):
        w1p[:, kh, :] = w1[kh * 128:(kh + 1) * 128, :]

    # heads with the pred layer folded in: Wc = w2 @ w_head [256, 256],
    # bias' = b2 @ w_head + b_head, consumed straight from h2 (K = 2x128 fp16)
    Wcr = w2 @ rw0
    Wco = w2 @ ow0
    brh = b2 @ rw0 + rb0
    boh = b2 @ ow0 + ob0
    wcr = np.zeros((128, 2, 256), np.float32)
    wco = np.zeros((128, 2, 256), np.float32)
    for kh in range(2):
        wcr[:, kh, :] = Wcr[kh * 128:(kh + 1) * 128, :]
        wco[:, kh, :] = Wco[kh * 128:(kh + 1) * 128, :]

    # routing out: DR over fp8 rhid stored at 512x -> rout_ps = 4096*routlin
    rw1dr = np.zeros((128, 2, 32), np.float32)
    for kt in range(2):
        rw1dr[:, kt, :] = 8.0 * rw1[kt * 128:(kt + 1) * 128, :]
    # offset out: fp16, x -> partition 0, y -> partition 32
    ow1p = np.zeros((128, 2, 64), np.float32)
    for kh in range(2):
        ow1p[:, kh, 0] = ow1[kh * 128:(kh + 1) * 128, 0]
        ow1p[:, kh, 32] = ow1[kh * 128:(kh + 1) * 128, 1]

    # static distance table for the offset horizontal band (b_off folded),
    # clipped to keep the fp16 eval in range (clipped taps evaluate to 0)
    ox = np.arange(256, dtype=np.float32)
    ixb = ox / 2.0 - 0.25 + 64.0 * b_off[0]
    dcol = np.clip(np.arange(W, dtype=np.float32)[:, None] - ixb[None, :], -8.0, 8.0)
    dstatp = np.tile(dcol, (1, 4))        # pair level [128, 1024]

    # vertical-weight selector; 1.25 compensates the Ky eval's 1/1.25
    sel = np.zeros((128, 32 * 128), np.float32)
    for half in range(2):
        for o in range(16):
            Sm = np.zeros((128, 128), np.float32)
            for m in range(128):
                r2 = half * 4 + m // 32
                Sm[o * 8 + r2, m] = 1.25
            sel[:, (half * 16 + o) * 128:(half * 16 + o + 1) * 128] = Sm

    sumc = np.zeros((128, 32), np.float32)
    sumc[np.arange(128), np.arange(128) % 32] = 1.0

    oybcast = np.zeros((16, 128), np.float32)
    oybcast[np.arange(128) // 8, np.arange(128)] = 1.0

    static = {
        "bandq8": _q8(bandq8.reshape(128, 2048)),
        "w0b": _q16(w0b),
        "w1p": _q16(w1p.reshape(128, 512)),
        "wcr": _q16(wcr.reshape(128, 512)),
        "wco": _q16(wco.reshape(128, 512)),
        "brha": _f32(RHS * brh[0:128].reshape(128, 1)),
        "brhb": _f32(RHS * brh[128:256].reshape(128, 1)),
        "boha": _f32(boh[0:128].reshape(128, 1)),
        "bohb": _f32(boh[128:256].reshape(128, 1)),
        "rw1dr": _q8(rw1dr.reshape(128, 64)),
        "ow1p": _q16(ow1p.reshape(128, 128)),
        "b1a": _f32(b1[0:128].reshape(128, 1)),
        "b1b": _f32(b1[128:256].reshape(128, 1)),
        "rb1p1": _f32((1.0 + rb1).reshape(32, 1)),
        "dstatp": _q16(dstatp),
        "sel": _q16(sel),
        "sumc16": _q16(sumc),
        "oybc64": _f32(64.0 * oybcast),
        "onesm": np.full((1, 128), -64.0, np.float32),
    }
    return {"rel": rel, "b_off": b_off, "static": static}


# ----------------------------------------------------------------------------
# per-core input tables
# ----------------------------------------------------------------------------

def _build_core_inputs(inputs, b, h, shared):
    x = np.asarray(inputs["x"])[b]
    x_loc = np.zeros((W, J, C), np.float32)
    rows = np.arange(J) + 64 * h - SHIFT
    valid = (rows >= 0) & (rows < H)
    x_loc[:, valid, :] = x[:, rows[valid], :].transpose(2, 1, 0)
    x_loc = x_loc.reshape(W, JC)

    b_off = shared["b_off"]
    # candcn[(o,r), g] = -(candpos - iy(oy) - 64*b_off_y); Act-bias form so
    # dy = 64*offy + candcn in one op (K is even in d; sentinel 16 -> K = 0)
    candcn = np.zeros((128, 8), np.float32)
    for g in range(8):
        for o in range(16):
            oy_g = h * 128 + 16 * g + o
            iy = oy_g / 2.0 - 0.25
            ky = int(np.floor(iy))
            for r in range(8):
                row = ky - 3 + r
                candcn[o * 8 + r, g] = -(
                    row - iy - 64.0 * b_off[1] if 0 <= row < H else 16.0)

    q0 = h * 128 * 256
    # kt rows 32..36 of the l1 input: 4 rel features + the bias-ones row
    relrows = np.zeros((5, OYS * 256), np.float32)
    relrows[0:4] = shared["rel"][q0:q0 + OYS * 256].T
    relrows[4] = 1.0

    d = {
        # 1.25 compensates the Kx eval's 1/1.25 scaling (see _keval)
        "x_loc16": _q16(1.25 * x_loc),
        "x_loc8f": _q8(x_loc),
        "candcn": _f32(candcn),
        "relrows16": _q16(relrows),
    }
    d.update(shared["static"])
    return d


_SHAPES = {
    "x_loc16": ((W, JC), FP16),
    "x_loc8f": ((W, JC), FP8),
    "candcn": ((128, 8), F32),
    "relrows16": ((5, OYS * 256), FP16),
    "bandq8": ((128, 2048), FP8),
    "w0b": ((37, 256), FP16),
    "w1p": ((128, 512), FP16),
    "wcr": ((128, 512), FP16),
    "wco": ((128, 512), FP16),
    "brha": ((128, 1), F32),
    "brhb": ((128, 1), F32),
    "boha": ((128, 1), F32),
    "bohb": ((128, 1), F32),
    "rw1dr": ((128, 64), FP8),
    "ow1p": ((128, 128), FP16),
    "b1a": ((128, 1), F32),
    "b1b": ((128, 1), F32),
    "rb1p1": ((32, 1), F32),
    "dstatp": ((128, 1024), FP16),
    "sel": ((128, 32 * 128), FP16),
    "sumc16": ((128, 32), FP16),
    "oybc64": ((16, 128), F32R),
    "onesm": ((1, 128), F32R),
}

# tiles reshaped to >2D on-chip
_TILE3D = {
    "x_loc8f": (W, J, C),
    "bandq8": (W, 2, 2, 2, 256),
    "w1p": (128, 2, 256),
    "wcr": (128, 2, 256),
    "wco": (128, 2, 256),
    "rw1dr": (128, 2, 32),
    "ow1p": (128, 2, 64),
}


def _fix_excess_waits(nc):
    """This walrus build allows only ONE semaphore wait per instruction.

    For any instruction carrying more, move the extra waits onto fresh NOPs
    inserted immediately before it on the same engine (identical semantics:
    the engine blocks on the same waits at the same program point).
    """
    blocks = list(nc.main_func.blocks)
    for bb in blocks:
        insts = bb.instructions
        i = 0
        while i < len(insts):
            ins = insts[i]
            si = ins.sync_info
            if si is not None and len(si.on_wait) > 1:
                waits = list(si.on_wait)
                extra, keep = waits[:-1], waits[-1:]
                nops = []
                for w in extra:
                    nop = nc.engines[ins.engine].nop(nofuse=True,
                                                     hint="wsplit").ins
                    for obb in blocks:
                        try:
                            obb.instructions.remove(nop)
                            break
                        except ValueError:
                            continue
                    nop.sync_info = mybir.SyncInfo(on_wait=[w], on_update=[])
                    nops.append(nop)
                ins.sync_info = mybir.SyncInfo(on_wait=keep,
                                               on_update=list(si.on_update))
                insts[i:i] = nops
                i += len(nops)
            i += 1


def _keval(nc, wp, et, nq, tag, out_bufs=2):
    """Exact bicubic kernel K(|et|)/1.25 -> fp16 tile [128, nq].

    K/1.25 = min(sa + (0.8 - 1.8 s), (-0.6 a + 0.6)(min(a,2)-2)^2), s = a^2,
    sa = s*a. Consumers' static weights carry the 1.25 back. Valid for the
    pre-clipped |et| <= ~16 range (fp16-safe).
    """
    a = wp.tile([128, nq], FP16, tag=f"{tag}a", bufs=3)
    s = wp.tile([128, nq], FP16, tag=f"{tag}s", bufs=3)
    c1 = wp.tile([128, nq], FP16, tag=f"{tag}c1", bufs=3)
    sa = wp.tile([128, nq], FP16, tag=f"{tag}sa", bufs=3)
    p1 = wp.tile([128, nq], FP16, tag=f"{tag}p1", bufs=3)
    t2 = wp.tile([128, nq], FP16, tag=f"{tag}t2", bufs=3)
    q2 = wp.tile([128, nq], FP16, tag=f"{tag}q2", bufs=3)
    r1 = wp.tile([128, nq], FP16, tag=f"{tag}r1", bufs=3)
    m2 = wp.tile([128, nq], FP16, tag=f"{tag}m2", bufs=3)
    kt = wp.tile([128, nq], FP16, tag=f"{tag}kt", bufs=out_bufs)
    nc.scalar.activation(a[:, :], et[:, :], AF.Abs)
    nc.vector.tensor_tensor(s[:, :], et[:, :], et[:, :], ALU.mult)
    nc.gpsimd.tensor_scalar(c1[:, :], s[:, :], -1.8, 0.8, ALU.mult, ALU.add)
    nc.vector.tensor_tensor(sa[:, :], s[:, :], a[:, :], ALU.mult)
    nc.vector.tensor_tensor(p1[:, :], sa[:, :], c1[:, :], ALU.add)
    nc.gpsimd.tensor_scalar(t2[:, :], a[:, :], -2.0, 0.0, ALU.add, ALU.min)
    nc.vector.tensor_tensor(q2[:, :], t2[:, :], t2[:, :], ALU.mult)
    nc.gpsimd.tensor_scalar(r1[:, :], a[:, :], -0.6, 0.6, ALU.mult, ALU.add)
    nc.vector.tensor_tensor(m2[:, :], r1[:, :], q2[:, :], ALU.mult)
    nc.vector.tensor_tensor(kt[:, :], p1[:, :], m2[:, :], ALU.min)
    return kt


def _build_program():
    nc = bass.Bass()
    P = {n: nc.declare_dram_parameter(n, list(s), d, isOutput=False)
         for n, (s, d) in _SHAPES.items()}
    outp = nc.declare_dram_parameter("outp", [C, OYS * 256], F32, isOutput=True)

    with tile.TileContext(nc) as tc:
        with (
            tc.tile_pool(name="consts", bufs=1) as cp,
            tc.tile_pool(name="work", bufs=2) as wp,
            tc.tile_pool(name="psM", bufs=2, space="PSUM") as psM,
            tc.tile_pool(name="psQ", bufs=2, space="PSUM") as psQ,
            tc.tile_pool(name="psG", bufs=2, space="PSUM") as psG,
        ):
            ct = {}
            for n, (s, d) in _SHAPES.items():
                if n == "relrows16":
                    continue  # streamed from DRAM per chunk
                shape = list(_TILE3D.get(n, s))
                t = cp.tile(shape, d, tag=n, name=n + "_sb")
                full = tuple(slice(None) for _ in shape)
                nc.gpsimd.dma_start(out=t[full], in_=P[n][:, :])
                ct[n] = t

            state = {}

            def chunk_mlp(g, cc):
                ch = g * 8 + cc
                sub = cc & 1
                oyA, oyB = 2 * ch, 2 * ch + 1
                y16 = state["y16"]

                # ---------- first sample: fused vertical x horizontal DR
                qf_ps = psQ.tile([32, 512], F32, tag="psQ", name="qf_ps")
                for t_i, oy in ((0, oyA), (1, oyB)):
                    sp = _s_prog(oy)
                    par = oy & 1
                    for rp in range(2):
                        j0 = sp + 2 + 2 * rp
                        nc.tensor.matmul(
                            qf_ps[:, t_i * 256:(t_i + 1) * 256],
                            ct["x_loc8f"][:, j0:j0 + 2, :],
                            ct["bandq8"][:, par, rp, :, :],
                            start=(rp == 0), stop=(rp == 1), perf_mode=DR)

                inp = wp.tile([37, 512], FP16, tag="inp", bufs=4,
                              name=f"inp_{ch}")
                nc.scalar.copy(inp[0:32, :], qf_ps[:, :])
                nc.sync.dma_start(
                    out=inp[32:37, :],
                    in_=P["relrows16"][:, ch * 512:(ch + 1) * 512])

                # ---------- MLP trunk (fp16)
                l1ps = psM.tile([128, 1024], F32, tag="psM", name="l1ps")
                for mh in range(2):
                    nc.tensor.matmul(l1ps[:, mh * 512:(mh + 1) * 512],
                                     ct["w0b"][:, mh * 128:(mh + 1) * 128],
                                     inp[:, :], start=True, stop=True)
                h1 = wp.tile([128, 2, 512], FP16, tag="h1", bufs=3, name="h1")
                nc.scalar.activation(h1[:, :, :], l1ps[:, :], AF.Relu)

                l2ps = psM.tile([128, 1024], F32, tag="psM", name="l2ps")
                for mh in range(2):
                    for kh in range(2):
                        nc.tensor.matmul(
                            l2ps[:, mh * 512:(mh + 1) * 512],
                            ct["w1p"][:, kh, mh * 128:(mh + 1) * 128],
                            h1[:, kh, :], start=(kh == 0), stop=(kh == 1))
                h2 = wp.tile([128, 2, 512], FP16, tag="h2", bufs=3, name="h2")
                nc.scalar.activation(h2[:, 0, :], l2ps[:, 0:512], AF.Relu,
                                     bias=ct["b1a"][:, 0:1])
                nc.vector.tensor_scalar(h2[:, 1, :], l2ps[:, 512:1024],
                                        ct["b1b"][:, 0:1], 0.0,
                                        ALU.add, ALU.max)

                # ---------- heads (pred layer folded into Wc = w2 @ w_head)
                rhps = psM.tile([128, 1024], F32, tag="psM", name="rhps")
                for mh in range(2):
                    for kh in range(2):
                        nc.tensor.matmul(
                            rhps[:, mh * 512:(mh + 1) * 512],
                            ct["wcr"][:, kh, mh * 128:(mh + 1) * 128],
                            h2[:, kh, :], start=(kh == 0), stop=(kh == 1))
                rhid = wp.tile([128, 2, 512], FP8, tag="rhid", bufs=3, name="rhid")
                nc.scalar.activation(rhid[:, 0, :], rhps[:, 0:512], AF.Relu,
                                     bias=ct["brha"][:, 0:1], scale=RHS)
                nc.scalar.activation(rhid[:, 1, :], rhps[:, 512:1024], AF.Relu,
                                     bias=ct["brhb"][:, 0:1], scale=RHS)

                ohps = psM.tile([128, 1024], F32, tag="psM", name="ohps")
                for mh in range(2):
                    for kh in range(2):
                        nc.tensor.matmul(
                            ohps[:, mh * 512:(mh + 1) * 512],
                            ct["wco"][:, kh, mh * 128:(mh + 1) * 128],
                            h2[:, kh, :], start=(kh == 0), stop=(kh == 1))
                ohid = wp.tile([128, 2, 512], FP16, tag="ohid", bufs=3, name="ohid")
                nc.scalar.activation(ohid[:, 0, :], ohps[:, 0:512], AF.Relu,
                                     bias=ct["boha"][:, 0:1])
                nc.vector.tensor_scalar(ohid[:, 1, :], ohps[:, 512:1024],
                                        ct["bohb"][:, 0:1], 0.0,
                                        ALU.add, ALU.max)

                rout_ps = psQ.tile([32, 512], F32, tag="psQ", name="rout_ps")
                nc.tensor.matmul(rout_ps[:, :], ct["rw1dr"][:, :, :],
                                 rhid[:, :, :], start=True, stop=True,
                                 perf_mode=DR)
                rpo = wp.tile([32, 512], FP16, tag="rpo", bufs=20,
                              name=f"rpo_{ch}")
                nc.scalar.activation(rpo[:, :], rout_ps[:, :], AF.Identity,
                                     bias=ct["rb1p1"][:, 0:1],
                                     scale=1.0 / 4096.0)
                state["rpo"][ch] = rpo

                off_ps = psQ.tile([64, 512], F32, tag="psQ", name="off_ps")
                for kh in range(2):
                    nc.tensor.matmul(off_ps[:, :], ct["ow1p"][:, kh, :],
                                     ohid[:, kh, :],
                                     start=(kh == 0), stop=(kh == 1))
                # raw offsets out of PSUM in one op (rows 0 = x, 32 = y)
                oxy = wp.tile([33, 512], F32R, tag="oxy", bufs=4, name="oxy")
                nc.scalar.copy(oxy[:, :], off_ps[0:33, :])
                nc.sync.dma_start(out=y16[2 * cc:2 * cc + 2, :],
                                  in_=oxy[32:33, :])

                # ---------- offset horizontal band (eval at chunk-pair level)
                bx_ps = psQ.tile([128, 512], F32, tag="psQ", name="bx_ps")
                nc.tensor.matmul(bx_ps[:, :], ct["onesm"][:, :], oxy[0:1, :],
                                 start=True, stop=True)
                if sub == 0:
                    state["et"] = wp.tile([128, 1024], FP16, tag="bxet",
                                          bufs=3, name="bxet")
                nc.vector.tensor_tensor(
                    state["et"][:, sub * 512:(sub + 1) * 512], bx_ps[:, :],
                    ct["dstatp"][:, 0:512], ALU.add)
                if sub == 1:
                    state["ktp"][ch // 2] = _keval(nc, wp, state["et"], 1024,
                                                   "bx", out_bufs=10)

            def group_cwy(g):
                y16 = state["y16"]
                cwyin = psQ.tile([128, 256], F32, tag="psQ", name="cwyin")
                nc.tensor.matmul(cwyin[:, :], ct["oybc64"][:, :], y16[:, :],
                                 start=True, stop=True)
                dy = wp.tile([128, 256], FP16, tag="cwdy", bufs=3, name="cwdy")
                nc.scalar.activation(dy[:, :], cwyin[:, :], AF.Identity,
                                     bias=ct["candcn"][:, g:g + 1])
                state["cwyp"][g] = _keval(nc, wp, dy, 256, "cw", out_bufs=3)

            def chunk_gather(g, cc):
                ch = g * 8 + cc
                sub = cc & 1
                oyA, oyB = 2 * ch, 2 * ch + 1
                spA, spB = _s_prog(oyA), _s_prog(oyB)
                ktp = state["ktp"][ch // 2]
                cwyp = state["cwyp"][g]

                out0 = psQ.tile([32, 512], F32, tag="psQ", name="out0")
                for half in range(2):
                    hgps = psG.tile([128, 512], F32, tag="psG", name="hgps")
                    for t_i, sp in ((0, spA), (1, spB)):
                        nc.tensor.matmul(
                            hgps[:, t_i * 256:t_i * 256 + 256],
                            ct["x_loc16"][:, sp * 32 + half * 128:
                                          sp * 32 + half * 128 + 128],
                            ktp[:, sub * 512 + t_i * 256:
                                sub * 512 + t_i * 256 + 256],
                            start=True, stop=True)
                    wyps = psG.tile([128, 512], F32, tag="psG", name="wyps")
                    for t_i in range(2):
                        o = (2 * cc + t_i) % 16
                        si = (half * 16 + o) * 128
                        nc.tensor.matmul(
                            wyps[:, t_i * 256:t_i * 256 + 256],
                            ct["sel"][:, si:si + 128],
                            cwyp[:, :], start=True, stop=True)
                    wys = wp.tile([128, 512], FP16, tag="wys", bufs=6,
                                  name="wys")
                    if half == 0:
                        nc.scalar.copy(wys[:, :], wyps[:, :])
                    else:
                        nc.vector.tensor_scalar(wys[:, :], wyps[:, :], 1.0,
                                                None, ALU.mult)
                    hwt = wp.tile([128, 512], FP16, tag="hwt", bufs=6,
                                  name="hwt")
                    nc.vector.tensor_tensor(hwt[:, :], hgps[:, :], wys[:, :],
                                            ALU.mult)
                    nc.tensor.matmul(
                        out0[:, :], ct["sumc16"][:, :], hwt[:, :],
                        start=(half == 0), stop=(half == 1))
                if sub == 0:
                    state["outtp"] = wp.tile([32, 1024], F32, tag="outtp", bufs=3,
                                             name="outtp")
                nc.vector.tensor_tensor(
                    state["outtp"][:, sub * 512:(sub + 1) * 512], out0[:, :],
                    state["rpo"][ch][:, :], ALU.mult)
                if sub == 1:
                    nc.sync.dma_start(
                        out=outp[:, (ch - 1) * 512:(ch + 1) * 512],
                        in_=state["outtp"][:, :])

            # software pipeline: group g's MLP interleaves with group g-1's
            # gather at chunk granularity
            state["rpo"] = {}
            state["ktp"] = {}
            state["cwyp"] = {}
            for g in range(N_GROUPS):
                state["y16"] = wp.tile([16, 256], F32R, tag="y16", bufs=3,
                                       name=f"y16_{g}")
                for cc in range(8):
                    chunk_mlp(g, cc)
                    if g > 0:
                        chunk_gather(g - 1, cc)
                group_cwy(g)
            for cc in range(8):
                chunk_gather(N_GROUPS - 1, cc)

    _fix_excess_waits(nc)
    return nc


_PROGRAM = None
_LAST_EXEC_NS = None


def kernel(**inputs):
    global _PROGRAM
    if _PROGRAM is None:
        _PROGRAM = _build_program()
    nc = _PROGRAM
    shared = _build_shared(inputs)
    in_maps = []
    for core in range(N_CORES):
        b, h = divmod(core, 2)
        in_maps.append(_build_core_inputs(inputs, b, h, shared))
    trace = os.environ.get("KTRACE", "0") == "1"
    try:
        res = run_bass_kernel_spmd(nc, in_maps, list(range(N_CORES)),
                                   trace=trace)
    except Exception:
        if not trace:
            raise
        res = run_bass_kernel_spmd(nc, in_maps, list(range(N_CORES)))
    global _LAST_EXEC_NS
    _LAST_EXEC_NS = res.exec_time_ns
    out = np.zeros((B, C, HO, WO), np.float32)
    for core in range(N_CORES):
        b, h = divmod(core, 2)
        o = res.results[core]["outp"].reshape(C, OYS, 256)
        out[b, :, h * 128:h * 128 + 128, :] = o
    return out
